# revision 1
# baseline (speedup 1.0000x reference)
"""Trainium2 Bass kernel for BatchRemoveQuatDiscontinuities.

Algorithm (per (batch, joint) lane):
    d[t]    = dot(q[t], q[t-1])                (fp32, 4-wide dot)
    flip[t] = 1 if d[t] < 0 else 0             (t >= 1; flip[0] = 0)
    sigma[t] = (-1)^(sum_{s<=t} flip[s])       (cumulative sign parity)
    out[t]  = q[t] * sigma[t]

Mapping on a NeuronCore (data-parallel over batch across 8 cores):
  * One tile = one batch clip, loaded as a single fully-contiguous 1MB
    DMA: [128 partitions = t/8, free = (ts: 8, j: 64, c: 4)].  Loads on
    the SP HWDGE ring (nc.sync), stores on the ACT ring (nc.scalar).
  * q[t-1]: within a partition it is a free-axis offset (-256); the
    octet boundary (ts=0) needs q[p-1, ts=7], produced by a TensorE
    matmul with an off-diagonal 0/1 matrix S into PSUM (fp32 exact).
  * prod on DVE, 4-wide dot via two pairwise adds (c0+c1)+(c2+c3),
    d written in (j, ts) order; flip indicator e = Relu(Sign(-d)) on
    ScalarE (bf16).
  * Within-octet inclusive prefix: tensor_tensor_scan with a reset mask
    (state = mask*state xor e), segments of 8 per joint.  Octet-level
    parity via strict-triangular matmul over partitions on the per-row
    totals; parity of the count -> sigo (int &1, ACT affine to +-1).
  * sigma_row = 1-2*rowp on ScalarE (bf16); sig = sigr*sigo on GpSimd.
    Final out = q * sig (broadcast over c), ts-split DVE/GpSimd by
    mult_split - exact +/-1 multiply.

Engine budget (HW-measured): DVE runs ~1.4ns/elem fp32 TT and is the
critical engine; GpSimd is ~4-6x slower than its cost model and only
tolerates ~16-24K elems/rep; ScalarE absorbs the activation chain; the
DMA floor (in+out on the two HWDGE rings) is ~103.5us/rep.
"""

import numpy as np
from contextlib import ExitStack

import concourse.bass as bass
import concourse.bacc as bacc
import concourse.tile as tile
from concourse import mybir
from concourse.bass_utils import run_bass_kernel_spmd

B, T, J, C = 128, 1024, 64, 4
NCORES = 8
JC = J * C                      # 256 floats per t
BPC = B // NCORES               # 16 batch clips per core
TS = 8                          # t per partition (octet)
FD = TS * JC                    # tile free dim = 2048 floats
SD = J * TS                     # prefix free dim = 512 (j, ts)

FP32 = mybir.dt.float32
BF16 = mybir.dt.bfloat16
I32 = mybir.dt.int32
Alu = mybir.AluOpType
Act = mybir.ActivationFunctionType


def _ap(apx, dims):
    """AP with explicit [step, count] free dims appended to partition dim."""
    return bass.AP(
        tensor=apx.tensor, offset=apx.offset,
        ap=[list(apx.ap[0]), *[list(d) for d in dims]],
    )


def build_nc(bpc=BPC, t=T, reps=1, mode="full", mult_split=4,
             sig_eng="dve", d_eng="dve", u_eng="dve", scan_eng="dve",
             out_ring="sync", qbufs=8, obufs=5, sbufs=4, sigo_cos=0,
             out_bf16=0, bobufs=5, cp=1):
    assert t % (128 * TS) == 0
    nc = bacc.Bacc(None, target_bir_lowering=False)
    q = nc.declare_dram_parameter("q", [bpc, t, J, C], FP32, isOutput=False)
    smat = nc.declare_dram_parameter("smat", [128, 128], FP32, isOutput=False)
    pmat = nc.declare_dram_parameter("pmat", [128, 128], FP32, isOutput=False)
    out_dt = BF16 if out_bf16 else FP32
    out = nc.declare_dram_parameter("out", [bpc, t, J, C], out_dt,
                                    isOutput=True)
    qf = q.rearrange("b t j c -> b (t j c)")
    of = out.rearrange("b t j c -> b (t j c)")

    eng_sig = nc.gpsimd if sig_eng == "pool" else nc.vector
    eng_d = nc.gpsimd if d_eng == "pool" else nc.vector
    eng_u = nc.gpsimd if u_eng == "pool" else nc.vector
    eng_scan = nc.gpsimd if scan_eng == "pool" else nc.vector
    eng_out = nc.scalar if out_ring == "act" else nc.sync

    with tile.TileContext(nc) as tc, ExitStack() as ctx:
        consts = ctx.enter_context(tc.tile_pool(name="consts", bufs=1))
        qpool = ctx.enter_context(tc.tile_pool(name="qpool", bufs=qbufs))
        opool = ctx.enter_context(tc.tile_pool(name="opool", bufs=obufs))
        bopool = (ctx.enter_context(tc.tile_pool(name="bopool", bufs=bobufs))
                  if out_bf16 else None)
        spool = ctx.enter_context(tc.tile_pool(name="spool", bufs=sbufs))
        auxp = ctx.enter_context(tc.tile_pool(name="auxp", bufs=4, space="PSUM"))
        offp = ctx.enter_context(tc.tile_pool(name="offp", bufs=4, space="PSUM"))

        smatSB = consts.tile([128, 128], FP32)
        nc.sync.dma_start(out=smatSB[:, :], in_=smat[:, :])
        pmatSB = consts.tile([128, 128], FP32)
        nc.sync.dma_start(out=pmatSB[:, :], in_=pmat[:, :])
        amask = consts.tile([128, cp * SD], FP32)
        nc.vector.memset(amask[:, :], 1.0)
        nc.vector.memset(
            amask.rearrange("p (b j ts) -> p b j ts", b=cp, ts=TS)[:, :, :, 0],
            0.0,
        )
        pihalf = consts.tile([128, 1], FP32)
        nc.vector.memset(pihalf[:, :], float(np.pi / 2))

        def emit_body():
            if cp == 1:
                for b in range(bpc):
                    emit_tile(b)
            else:
                for g in range(bpc // cp):
                    emit_tile_cp2(g)

        def emit_tile_cp2(g):
            """Two clips per tile: same per-engine work doses as the cp=1
            path (Pool only gets the final-mult halves), but the 1024-wide
            stages (prod/scan/sig/ACT chain) are fused across the clip pair
            to halve instruction counts, and the DMAs are 2MB."""
            qt = qpool.tile([128, cp, FD], FP32, tag="qt")
            nc.sync.dma_start(
                out=qt[:, :, :],
                in_=qf[g * cp:(g + 1) * cp, :].rearrange(
                    "b (p x) -> p b x", p=128),
            )
            o = opool.tile([128, cp, FD], FP32, tag="o")
            aux = auxp.tile([128, cp, JC], FP32, tag="aux")
            nc.tensor.matmul(
                aux[:, :, :], lhsT=smatSB[:, :], rhs=qt[:, :, FD - JC:FD],
                start=True, stop=True,
            )
            nc.vector.tensor_tensor(
                out=o[:, :, JC:FD], in0=qt[:, :, JC:FD],
                in1=qt[:, :, 0:FD - JC], op=Alu.mult,
            )
            nc.vector.tensor_tensor(
                out=o[:, :, 0:JC], in0=qt[:, :, 0:JC], in1=aux[:, :, :],
                op=Alu.mult,
            )
            u = spool.tile([128, cp, 2 * SD], FP32, tag="u")
            d = spool.tile([128, cp, SD], FP32, tag="d")
            for c2 in range(cp):
                opairs = o.rearrange(
                    "p b (s k two) -> p b s k two", k=2, two=2)[:, c2]
                uv = u.rearrange("p b (s k) -> p b s k", k=2)[:, c2]
                nc.vector.tensor_tensor(
                    out=uv, in0=opairs[:, :, :, 0], in1=opairs[:, :, :, 1],
                    op=Alu.add,
                )
                u_k = u.rearrange(
                    "p b (ts j k) -> p b ts j k", j=J, k=2)[:, c2]
                dv = bass.AP(
                    tensor=d.tensor, offset=d.offset + c2 * SD,
                    ap=[list(d.ap[0]), [1, TS], [TS, J]],
                )
                nc.vector.tensor_tensor(
                    out=dv, in0=u_k[:, :, :, 0], in1=u_k[:, :, :, 1],
                    op=Alu.add,
                )
            df = d.rearrange("p b s -> p (b s)")
            sg = spool.tile([128, cp * SD], FP32, tag="sg")
            nc.scalar.activation(sg[:, :], df, Act.Sign, scale=-1.0)
            e = spool.tile([128, cp * SD], BF16, tag="e")
            nc.scalar.activation(e[:, :], sg[:, :], Act.Relu)
            ev = e.rearrange("p (b j ts) -> p b j ts", b=cp, ts=TS)
            nc.scalar.mul(ev[0:1, :, :, 0], ev[0:1, :, :, 0], 0.0)
            rowp = spool.tile([128, cp * SD], FP32, tag="rowp")
            nc.vector.tensor_tensor_scan(
                out=rowp[:, :], data0=amask[:, :], data1=e[:, :],
                initial=0.0, op0=Alu.mult, op1=Alu.logical_xor,
            )
            offs = offp.tile([128, cp, J], FP32, tag="offs")
            rr = rowp.rearrange("p (b j ts) -> p b j ts", b=cp, ts=TS)
            nc.tensor.matmul(
                offs[:, :, :], lhsT=pmatSB[:, :], rhs=rr[:, :, :, 7],
                start=True, stop=True,
            )
            offi = spool.tile([128, cp * J], I32, tag="offi")
            nc.vector.tensor_copy(
                out=offi[:, :], in_=offs.rearrange("p b j -> p (b j)"))
            offb = spool.tile([128, cp * J], I32, tag="offb")
            nc.vector.tensor_scalar(
                out=offb[:, :], in0=offi[:, :], scalar1=1, scalar2=None,
                op0=Alu.bitwise_and,
            )
            sigo = spool.tile([128, cp * J], BF16, tag="sigo")
            nc.scalar.activation(sigo[:, :], offb[:, :], Act.Copy,
                                 bias=1.0, scale=-2.0)
            sigr = spool.tile([128, cp * SD], BF16, tag="sigr")
            nc.scalar.activation(sigr[:, :], rowp[:, :], Act.Copy,
                                 bias=1.0, scale=-2.0)
            sig = spool.tile([128, cp * SD], BF16, tag="sig")
            nc.vector.tensor_tensor(
                out=sig.rearrange("p (bj ts) -> p bj ts", ts=TS),
                in0=sigr.rearrange("p (bj ts) -> p bj ts", ts=TS),
                in1=_ap(sigo, [[1, cp * J], [0, TS]]),
                op=Alu.mult,
            )
            for c2 in range(cp):
                qv = qt.rearrange("p b (ts x) -> p b ts x", ts=TS)[:, c2]
                ow = o.rearrange("p b (ts x) -> p b ts x", ts=TS)[:, c2]
                sbase = sig.offset + c2 * SD
                if mult_split > 0:
                    nc.vector.tensor_tensor(
                        out=ow[:, 0:mult_split, :],
                        in0=qv[:, 0:mult_split, :],
                        in1=bass.AP(
                            tensor=sig.tensor, offset=sbase,
                            ap=[list(sig.ap[0]), [1, mult_split], [TS, J],
                                [0, C]],
                        ),
                        op=Alu.mult,
                    )
                if mult_split < TS:
                    nc.gpsimd.tensor_tensor(
                        out=ow[:, mult_split:TS, :],
                        in0=qv[:, mult_split:TS, :],
                        in1=bass.AP(
                            tensor=sig.tensor, offset=sbase + mult_split,
                            ap=[list(sig.ap[0]), [1, TS - mult_split],
                                [TS, J], [0, C]],
                        ),
                        op=Alu.mult,
                    )
            eng_out.dma_start(
                out=of[g * cp:(g + 1) * cp, :].rearrange(
                    "b (p x) -> p b x", p=128),
                in_=o[:, :, :],
            )

        def emit_tile(b):
            qt = qpool.tile([128, FD], FP32, tag="qt")
            nc.sync.dma_start(
                out=qt[:, :],
                in_=qf[b, :].rearrange("(p x) -> p x", p=128),
            )
            o = opool.tile([128, FD], FP32, tag="o")
            if mode == "dma":
                eng_out.dma_start(
                    out=of[b, :].rearrange("(p x) -> p x", p=128), in_=qt[:, :]
                )
                return

            # octet-boundary shift: aux[p] = qt[p-1, ts=7 chunk] (row 0 = 0)
            aux = auxp.tile([128, JC], FP32, tag="aux")
            nc.tensor.matmul(
                aux[:, :],
                lhsT=smatSB[:, :],
                rhs=qt[:, FD - JC:FD],
                start=True,
                stop=True,
            )

            # prod: o = q * q_shifted  (DVE)
            nc.vector.tensor_tensor(
                out=o[:, JC:FD], in0=qt[:, JC:FD], in1=qt[:, 0:FD - JC],
                op=Alu.mult,
            )
            nc.vector.tensor_tensor(
                out=o[:, 0:JC], in0=qt[:, 0:JC], in1=aux[:, :], op=Alu.mult,
            )

            # dot over c, pairwise (c0+c1)+(c2+c3); d written in (j, ts) order
            u = spool.tile([128, 2 * SD], FP32, tag="u")
            ov = o.rearrange("p (s c) -> p s c", c=C)
            uv = u.rearrange("p (s k) -> p s k", k=2)
            opairs = ov.rearrange("p s (k two) -> p s k two", k=2)
            eng_u.tensor_tensor(
                out=uv, in0=opairs[:, :, :, 0], in1=opairs[:, :, :, 1],
                op=Alu.add,
            )
            d = spool.tile([128, SD], FP32, tag="d")  # (j, ts) layout
            u_k = u.rearrange("p (ts j k) -> p ts j k", j=J, k=2)
            eng_d.tensor_tensor(
                out=_ap(d, [[1, TS], [TS, J]]),
                in0=u_k[:, :, :, 0],
                in1=u_k[:, :, :, 1],
                op=Alu.add,
            )

            # flip indicator e = Relu(Sign(-d)), bf16, (j, ts) layout  (ACT)
            sg = spool.tile([128, SD], FP32, tag="sg")
            nc.scalar.activation(sg[:, :], d[:, :], Act.Sign, scale=-1.0)
            e = spool.tile([128, SD], BF16, tag="e")
            nc.scalar.activation(e[:, :], sg[:, :], Act.Relu)
            # t=0 has no flip (also guards Sign(0) semantics)
            nc.scalar.mul(
                e.rearrange("p (j ts) -> p j ts", ts=TS)[0:1, :, 0],
                e.rearrange("p (j ts) -> p j ts", ts=TS)[0:1, :, 0],
                0.0,
            )

            # within-octet inclusive prefix PARITY (segmented xor-scan):
            # state = (mask * state) xor e  -> 0/1 running parity per joint
            rowp = spool.tile([128, SD], FP32, tag="rowp")
            eng_scan.tensor_tensor_scan(
                out=rowp[:, :], data0=amask[:, :], data1=e[:, :],
                initial=0.0, op0=Alu.mult, op1=Alu.logical_xor,
            )

            # octet-level: count of odd rows above (parity-sum via matmul)
            offs = offp.tile([128, J], FP32, tag="offs")
            nc.tensor.matmul(
                offs[:, :],
                lhsT=pmatSB[:, :],
                rhs=rowp.rearrange("p (j ts) -> p j ts", ts=TS)[:, :, 7],
                start=True,
                stop=True,
            )
            # parity of that count -> sigma_off in {+1, -1} per (p, j)
            sigo = spool.tile([128, J], BF16, tag="sigo")
            if sigo_cos:
                # count mod 2 on DVE (exact: counts are integer-valued fp32),
                # then the +-1 affine on ACT.
                offm = spool.tile([128, J], FP32, tag="offm")
                nc.vector.tensor_scalar(
                    out=offm[:, :], in0=offs[:, :], scalar1=2.0, scalar2=None,
                    op0=Alu.mod,
                )
                nc.scalar.activation(sigo[:, :], offm[:, :], Act.Copy,
                                     bias=1.0, scale=-2.0)
            else:
                offi = spool.tile([128, J], I32, tag="offi")
                nc.vector.tensor_copy(out=offi[:, :], in_=offs[:, :])
                offb = spool.tile([128, J], I32, tag="offb")
                nc.vector.tensor_scalar(
                    out=offb[:, :], in0=offi[:, :], scalar1=1, scalar2=None,
                    op0=Alu.bitwise_and,
                )
                nc.scalar.activation(sigo[:, :], offb[:, :], Act.Copy,
                                     bias=1.0, scale=-2.0)
            # sigma_row in {+1, -1} from the 0/1 row parity  (ACT)
            sigr = spool.tile([128, SD], BF16, tag="sigr")
            nc.scalar.activation(sigr[:, :], rowp[:, :], Act.Copy,
                                 bias=1.0, scale=-2.0)
            # sigma = sigma_row * sigma_off, (j, ts) layout
            sig = spool.tile([128, SD], BF16, tag="sig")
            eng_sig.tensor_tensor(
                out=sig.rearrange("p (j ts) -> p j ts", ts=TS),
                in0=sigr.rearrange("p (j ts) -> p j ts", ts=TS),
                in1=_ap(sigo, [[1, J], [0, TS]]),
                op=Alu.mult,
            )

            # out = q * sigma (broadcast over c), exact +/-1 multiply;
            # split by ts-range between VectorE and GpSimd.  With out_bf16
            # the result is written (rounded) to a bf16 tile and the store
            # moves half the bytes.
            if out_bf16:
                dst = bopool.tile([128, FD], BF16, tag="ob")
            else:
                dst = o
            qv = qt.rearrange("p (ts x) -> p ts x", ts=TS)
            ow = dst.rearrange("p (ts x) -> p ts x", ts=TS)
            tsplit = mult_split
            if tsplit > 0:
                nc.vector.tensor_tensor(
                    out=ow[:, 0:tsplit, :],
                    in0=qv[:, 0:tsplit, :],
                    in1=bass.AP(
                        tensor=sig.tensor, offset=sig.offset,
                        ap=[list(sig.ap[0]), [1, tsplit], [TS, J], [0, C]],
                    ),
                    op=Alu.mult,
                )
            if tsplit < TS:
                nc.gpsimd.tensor_tensor(
                    out=ow[:, tsplit:TS, :],
                    in0=qv[:, tsplit:TS, :],
                    in1=bass.AP(
                        tensor=sig.tensor, offset=sig.offset + tsplit,
                        ap=[list(sig.ap[0]), [1, TS - tsplit], [TS, J],
                            [0, C]],
                    ),
                    op=Alu.mult,
                )

            eng_out.dma_start(
                out=of[b, :].rearrange("(p x) -> p x", p=128), in_=dst[:, :]
            )

        if reps == 1:
            emit_body()
        else:
            with tc.For_i(0, reps, 1):
                emit_body()
    return nc


def make_consts():
    smat = np.eye(128, k=1, dtype=np.float32)       # S[k, m] = 1 iff m == k+1
    pmat = np.triu(np.ones((128, 128), np.float32), k=1)  # strict prefix
    return smat, pmat


def make_in_maps(q, smat, pmat):
    return [
        {"q": q[c * BPC:(c + 1) * BPC], "smat": smat, "pmat": pmat}
        for c in range(NCORES)
    ]


def kernel(joint_rotations: np.ndarray) -> np.ndarray:
    q = np.ascontiguousarray(joint_rotations, dtype=np.float32)
    assert q.shape == (B, T, J, C)
    smat, pmat = make_consts()
    nc = build_nc()
    nc.finalize()   # run bacc passes (wait splitting, reg alloc) + freeze
    in_maps = make_in_maps(q, smat, pmat)
    res = run_bass_kernel_spmd(nc, in_maps, list(range(NCORES)))
    outs = [np.asarray(r["out"]).astype(np.float32) for r in res.results]
    return np.concatenate(outs, axis=0)



# revision 20
# speedup vs baseline: 2.8023x; 2.8023x over previous
"""Trainium2 Bass kernel for BatchRemoveQuatDiscontinuities.

Algorithm (per (batch, joint) lane):
    d[t]    = dot(q[t], q[t-1])                (fp32, 4-wide dot)
    flip[t] = 1 if d[t] < 0 else 0             (t >= 1; flip[0] = 0)
    sigma[t] = (-1)^(sum_{s<=t} flip[s])       (cumulative sign parity)
    out[t]  = q[t] * sigma[t]

Mapping on a NeuronCore (data-parallel over batch across 8 cores):
  * One tile = one batch clip, loaded as a single fully-contiguous 1MB
    DMA: [128 partitions = t/8, free = (ts: 8, j: 64, c: 4)].  Loads on
    the SP HWDGE ring (nc.sync), stores on the ACT ring (nc.scalar).
  * q[t-1]: within a partition it is a free-axis offset (-256); the
    octet boundary (ts=0) needs q[p-1, ts=7], produced by a TensorE
    matmul with an off-diagonal 0/1 matrix S into PSUM (fp32 exact).
  * prod on DVE, 4-wide dot via two pairwise adds (c0+c1)+(c2+c3),
    d written in (j, ts) order; flip indicator e = Relu(Sign(-d)) on
    ScalarE (bf16).
  * Within-octet inclusive prefix: tensor_tensor_scan with a reset mask
    (state = mask*state xor e), segments of 8 per joint.  Octet-level
    parity via strict-triangular matmul over partitions on the per-row
    totals; parity of the count -> sigo (int &1, ACT affine to +-1).
  * sigma_row = 1-2*rowp on ScalarE (bf16); sig = sigr*sigo on GpSimd.
    Final out = q * sig (broadcast over c), ts-split DVE/GpSimd by
    mult_split - exact +/-1 multiply.

Engine budget (HW-measured, ablation-profiled): DVE is 100% critical at
~1.1-1.45ns/free-elem fp32 (prod 36us + u/d 36us + scan 11us + final
mult 36us per rep of 16 clips); 16-bit gives NO 2x here (broadcast
in1 AP blocks perf mode).  GpSimd TT hurts at ANY dose (ms7 = +4us) -
keep Pool idle.  ScalarE has ~60us slack and absorbs the bf16 cast.
DMA: load-only 45.6us/16MB (351GB/s), aggregate ~330GB/s/core; at 24MB
(fp32 in + bf16 out) DMA is NOT binding.  tensor_reduce(X) runs at
input rate (no win over pairwise adds); scalar_tensor_tensor is_gt is
~4x slower than plain TT (don't fuse e); tensor_tensor_scan does not
lower on Pool.  tc.For_i costs ~14us/iteration (all-engine barrier at
the back edge; staggered_reset no help) - amortize with loop_unroll.
Best config: out_bf16=1 cast_act=1 mult_split=8 (body ~134us vs 165us
for the old ms4 config, which was Pool-bound at 16K elems ~ 10ns/elem).
"""

import numpy as np
from contextlib import ExitStack

import concourse.bass as bass
import concourse.bacc as bacc
import concourse.tile as tile
from concourse import mybir
from concourse.bass_utils import run_bass_kernel_spmd

B, T, J, C = 128, 1024, 64, 4
NCORES = 8
JC = J * C                      # 256 floats per t
BPC = B // NCORES               # 16 batch clips per core
TS = 8                          # t per partition (octet)
FD = TS * JC                    # tile free dim = 2048 floats
SD = J * TS                     # prefix free dim = 512 (j, ts)

FP32 = mybir.dt.float32
BF16 = mybir.dt.bfloat16
I32 = mybir.dt.int32
Alu = mybir.AluOpType
Act = mybir.ActivationFunctionType


def _ap(apx, dims):
    """AP with explicit [step, count] free dims appended to partition dim."""
    return bass.AP(
        tensor=apx.tensor, offset=apx.offset,
        ap=[list(apx.ap[0]), *[list(d) for d in dims]],
    )


def build_nc(bpc=BPC, t=T, reps=1, mode="full", mult_split=8,
             sig_eng="dve", d_eng="dve", u_eng="dve", scan_eng="dve",
             out_ring="sync", qbufs=8, obufs=5, sbufs=4, sigo_cos=0,
             out_bf16=1, bobufs=5, cp=1, unroll=0, loop_unroll=1,
             staggered=0, fuse_e=0, e_eng="dve", cast_act=1, u_pool=0,
             probe="none", dred=0):
    assert t % (128 * TS) == 0
    nc = bacc.Bacc(None, target_bir_lowering=False)
    q = nc.declare_dram_parameter("q", [bpc, t, J, C], FP32, isOutput=False)
    smat = nc.declare_dram_parameter("smat", [128, 128], FP32, isOutput=False)
    pmat = nc.declare_dram_parameter("pmat", [128, 128], FP32, isOutput=False)
    out_dt = BF16 if out_bf16 else FP32
    out = nc.declare_dram_parameter("out", [bpc, t, J, C], out_dt,
                                    isOutput=True)
    qf = q.rearrange("b t j c -> b (t j c)")
    of = out.rearrange("b t j c -> b (t j c)")

    eng_sig = nc.gpsimd if sig_eng == "pool" else nc.vector
    eng_d = nc.gpsimd if d_eng == "pool" else nc.vector
    eng_u = nc.gpsimd if u_eng == "pool" else nc.vector
    eng_scan = nc.gpsimd if scan_eng == "pool" else nc.vector
    eng_e = nc.gpsimd if e_eng == "pool" else nc.vector
    eng_out = nc.scalar if out_ring == "act" else nc.sync

    with tile.TileContext(nc) as tc, ExitStack() as ctx:
        consts = ctx.enter_context(tc.tile_pool(name="consts", bufs=1))
        qpool = ctx.enter_context(tc.tile_pool(name="qpool", bufs=qbufs))
        opool = ctx.enter_context(tc.tile_pool(name="opool", bufs=obufs))
        bopool = (ctx.enter_context(tc.tile_pool(name="bopool", bufs=bobufs))
                  if out_bf16 else None)
        spool = ctx.enter_context(tc.tile_pool(name="spool", bufs=sbufs))
        auxp = ctx.enter_context(tc.tile_pool(name="auxp", bufs=4, space="PSUM"))
        offp = ctx.enter_context(tc.tile_pool(name="offp", bufs=4, space="PSUM"))

        smatSB = consts.tile([128, 128], FP32)
        nc.sync.dma_start(out=smatSB[:, :], in_=smat[:, :])
        pmatSB = consts.tile([128, 128], FP32)
        nc.sync.dma_start(out=pmatSB[:, :], in_=pmat[:, :])
        amask = consts.tile([128, cp * SD], FP32)
        nc.vector.memset(amask[:, :], 1.0)
        nc.vector.memset(
            amask.rearrange("p (b j ts) -> p b j ts", b=cp, ts=TS)[:, :, :, 0],
            0.0,
        )
        pihalf = consts.tile([128, 1], FP32)
        nc.vector.memset(pihalf[:, :], float(np.pi / 2))

        def emit_body():
            if cp == 1:
                for b in range(bpc):
                    emit_tile(b)
            else:
                for g in range(bpc // cp):
                    emit_tile_cp2(g)

        def emit_tile_cp2(g):
            """Two clips per tile: same per-engine work doses as the cp=1
            path (Pool only gets the final-mult halves), but the 1024-wide
            stages (prod/scan/sig/ACT chain) are fused across the clip pair
            to halve instruction counts, and the DMAs are 2MB."""
            qt = qpool.tile([128, cp, FD], FP32, tag="qt")
            nc.sync.dma_start(
                out=qt[:, :, :],
                in_=qf[g * cp:(g + 1) * cp, :].rearrange(
                    "b (p x) -> p b x", p=128),
            )
            o = opool.tile([128, cp, FD], FP32, tag="o")
            aux = auxp.tile([128, cp, JC], FP32, tag="aux")
            nc.tensor.matmul(
                aux[:, :, :], lhsT=smatSB[:, :], rhs=qt[:, :, FD - JC:FD],
                start=True, stop=True,
            )
            nc.vector.tensor_tensor(
                out=o[:, :, JC:FD], in0=qt[:, :, JC:FD],
                in1=qt[:, :, 0:FD - JC], op=Alu.mult,
            )
            nc.vector.tensor_tensor(
                out=o[:, :, 0:JC], in0=qt[:, :, 0:JC], in1=aux[:, :, :],
                op=Alu.mult,
            )
            u = spool.tile([128, cp, 2 * SD], FP32, tag="u")
            d = spool.tile([128, cp, SD], FP32, tag="d")
            for c2 in range(cp):
                opairs = o.rearrange(
                    "p b (s k two) -> p b s k two", k=2, two=2)[:, c2]
                uv = u.rearrange("p b (s k) -> p b s k", k=2)[:, c2]
                nc.vector.tensor_tensor(
                    out=uv, in0=opairs[:, :, :, 0], in1=opairs[:, :, :, 1],
                    op=Alu.add,
                )
                u_k = u.rearrange(
                    "p b (ts j k) -> p b ts j k", j=J, k=2)[:, c2]
                dv = bass.AP(
                    tensor=d.tensor, offset=d.offset + c2 * SD,
                    ap=[list(d.ap[0]), [1, TS], [TS, J]],
                )
                nc.vector.tensor_tensor(
                    out=dv, in0=u_k[:, :, :, 0], in1=u_k[:, :, :, 1],
                    op=Alu.add,
                )
            df = d.rearrange("p b s -> p (b s)")
            sg = spool.tile([128, cp * SD], FP32, tag="sg")
            nc.scalar.activation(sg[:, :], df, Act.Sign, scale=-1.0)
            e = spool.tile([128, cp * SD], BF16, tag="e")
            nc.scalar.activation(e[:, :], sg[:, :], Act.Relu)
            ev = e.rearrange("p (b j ts) -> p b j ts", b=cp, ts=TS)
            nc.scalar.mul(ev[0:1, :, :, 0], ev[0:1, :, :, 0], 0.0)
            rowp = spool.tile([128, cp * SD], FP32, tag="rowp")
            nc.vector.tensor_tensor_scan(
                out=rowp[:, :], data0=amask[:, :], data1=e[:, :],
                initial=0.0, op0=Alu.mult, op1=Alu.logical_xor,
            )
            offs = offp.tile([128, cp, J], FP32, tag="offs")
            rr = rowp.rearrange("p (b j ts) -> p b j ts", b=cp, ts=TS)
            nc.tensor.matmul(
                offs[:, :, :], lhsT=pmatSB[:, :], rhs=rr[:, :, :, 7],
                start=True, stop=True,
            )
            offi = spool.tile([128, cp * J], I32, tag="offi")
            nc.vector.tensor_copy(
                out=offi[:, :], in_=offs.rearrange("p b j -> p (b j)"))
            offb = spool.tile([128, cp * J], I32, tag="offb")
            nc.vector.tensor_scalar(
                out=offb[:, :], in0=offi[:, :], scalar1=1, scalar2=None,
                op0=Alu.bitwise_and,
            )
            sigo = spool.tile([128, cp * J], BF16, tag="sigo")
            nc.scalar.activation(sigo[:, :], offb[:, :], Act.Copy,
                                 bias=1.0, scale=-2.0)
            sigr = spool.tile([128, cp * SD], BF16, tag="sigr")
            nc.scalar.activation(sigr[:, :], rowp[:, :], Act.Copy,
                                 bias=1.0, scale=-2.0)
            sig = spool.tile([128, cp * SD], BF16, tag="sig")
            nc.vector.tensor_tensor(
                out=sig.rearrange("p (bj ts) -> p bj ts", ts=TS),
                in0=sigr.rearrange("p (bj ts) -> p bj ts", ts=TS),
                in1=_ap(sigo, [[1, cp * J], [0, TS]]),
                op=Alu.mult,
            )
            if out_bf16:
                dst = bopool.tile([128, cp, FD], BF16, tag="ob")
            else:
                dst = o
            if out_bf16 and cast_act:
                qb = bopool.tile([128, cp, FD], BF16, tag="qb")
                nc.scalar.copy(qb[:, :, :], qt[:, :, :])
                qsrc = qb
            else:
                qsrc = qt
            for c2 in range(cp):
                qv = qsrc.rearrange("p b (ts x) -> p b ts x", ts=TS)[:, c2]
                ow = dst.rearrange("p b (ts x) -> p b ts x", ts=TS)[:, c2]
                sbase = sig.offset + c2 * SD
                if mult_split > 0:
                    nc.vector.tensor_tensor(
                        out=ow[:, 0:mult_split, :],
                        in0=qv[:, 0:mult_split, :],
                        in1=bass.AP(
                            tensor=sig.tensor, offset=sbase,
                            ap=[list(sig.ap[0]), [1, mult_split], [TS, J],
                                [0, C]],
                        ),
                        op=Alu.mult,
                    )
                if mult_split < TS:
                    nc.gpsimd.tensor_tensor(
                        out=ow[:, mult_split:TS, :],
                        in0=qv[:, mult_split:TS, :],
                        in1=bass.AP(
                            tensor=sig.tensor, offset=sbase + mult_split,
                            ap=[list(sig.ap[0]), [1, TS - mult_split],
                                [TS, J], [0, C]],
                        ),
                        op=Alu.mult,
                    )
            eng_out.dma_start(
                out=of[g * cp:(g + 1) * cp, :].rearrange(
                    "b (p x) -> p b x", p=128),
                in_=dst[:, :, :],
            )

        stile = None
        if mode == "store":
            stile = consts.tile([128, FD], FP32)
            nc.vector.memset(stile[:, :], 1.0)

        def emit_tile(b):
            if mode == "store":
                eng_out.dma_start(
                    out=of[b, :].rearrange("(p x) -> p x", p=128),
                    in_=stile[:, :],
                )
                return
            qt = qpool.tile([128, FD], FP32, tag="qt")
            nc.sync.dma_start(
                out=qt[:, :],
                in_=qf[b, :].rearrange("(p x) -> p x", p=128),
            )
            if mode == "load":
                return
            o = opool.tile([128, FD], FP32, tag="o")
            if mode == "dma":
                eng_out.dma_start(
                    out=of[b, :].rearrange("(p x) -> p x", p=128), in_=qt[:, :]
                )
                return

            # octet-boundary shift: aux[p] = qt[p-1, ts=7 chunk] (row 0 = 0)
            aux = auxp.tile([128, JC], FP32, tag="aux")
            nc.tensor.matmul(
                aux[:, :],
                lhsT=smatSB[:, :],
                rhs=qt[:, FD - JC:FD],
                start=True,
                stop=True,
            )

            # prod: o = q * q_shifted  (DVE)
            if probe != "noprod":
                nc.vector.tensor_tensor(
                    out=o[:, JC:FD], in0=qt[:, JC:FD], in1=qt[:, 0:FD - JC],
                    op=Alu.mult,
                )
                nc.vector.tensor_tensor(
                    out=o[:, 0:JC], in0=qt[:, 0:JC], in1=aux[:, :],
                    op=Alu.mult,
                )
                osrc = o
            else:
                osrc = qt

            if dred:
                # d = reduce over c in one DVE pass, written in (j, ts) order
                d = spool.tile([128, SD], FP32, tag="d")
                nc.vector.tensor_reduce(
                    out=_ap(d, [[1, TS], [TS, J]]),
                    in_=osrc.rearrange("p (ts j c) -> p ts j c", j=J, c=C),
                    axis=mybir.AxisListType.X, op=Alu.add,
                )
                sg = spool.tile([128, SD], FP32, tag="sg")
                nc.scalar.activation(sg[:, :], d[:, :], Act.Sign, scale=-1.0)
                e = spool.tile([128, SD], BF16, tag="e")
                nc.scalar.activation(e[:, :], sg[:, :], Act.Relu)
            if not dred:
                # dot over c, pairwise (c0+c1)+(c2+c3); d in (j, ts) order
                u = spool.tile([128, 2 * SD], FP32, tag="u")
                ov = osrc.rearrange("p (s c) -> p s c", c=C)
                uv = u.rearrange("p (s k) -> p s k", k=2)
                opairs = ov.rearrange("p s (k two) -> p s k two", k=2)
                if probe == "noud":
                    pass
                elif u_pool:
                    nc.gpsimd.tensor_tensor(
                        out=uv[:, 0:u_pool], in0=opairs[:, 0:u_pool, :, 0],
                        in1=opairs[:, 0:u_pool, :, 1], op=Alu.add,
                    )
                    eng_u.tensor_tensor(
                        out=uv[:, u_pool:SD], in0=opairs[:, u_pool:SD, :, 0],
                        in1=opairs[:, u_pool:SD, :, 1], op=Alu.add,
                    )
                else:
                    eng_u.tensor_tensor(
                        out=uv, in0=opairs[:, :, :, 0], in1=opairs[:, :, :, 1],
                        op=Alu.add,
                    )
                u_k = u.rearrange("p (ts j k) -> p ts j k", j=J, k=2)
            if dred:
                pass
            elif probe == "noud":
                d = spool.tile([128, SD], FP32, tag="d")
                sg = spool.tile([128, SD], FP32, tag="sg")
                nc.scalar.activation(sg[:, :], amask[:, 0:SD], Act.Sign,
                                     scale=-1.0)
                e = spool.tile([128, SD], BF16, tag="e")
                nc.scalar.activation(e[:, :], sg[:, :], Act.Relu)
            elif fuse_e:
                # e = 1.0 iff (-u0 > u1) <=> d = u0 + u1 < 0, one DVE/Pool
                # op replacing the d add + ACT Sign + ACT Relu chain.  The
                # comparison of the two fp values is exact, and for near-tie
                # sums fl(u0+u1) is exact (Sterbenz), so this matches the
                # old fl(d) < 0 test bit-for-bit.
                e = spool.tile([128, SD], BF16, tag="e")
                eng_e.scalar_tensor_tensor(
                    out=_ap(e, [[1, TS], [TS, J]]),
                    in0=u_k[:, :, :, 0], scalar=-1.0, in1=u_k[:, :, :, 1],
                    op0=Alu.mult, op1=Alu.is_gt,
                )
            else:
                d = spool.tile([128, SD], FP32, tag="d")  # (j, ts) layout
                eng_d.tensor_tensor(
                    out=_ap(d, [[1, TS], [TS, J]]),
                    in0=u_k[:, :, :, 0],
                    in1=u_k[:, :, :, 1],
                    op=Alu.add,
                )

                # flip indicator e = Relu(Sign(-d)), bf16, (j, ts) layout
                sg = spool.tile([128, SD], FP32, tag="sg")
                nc.scalar.activation(sg[:, :], d[:, :], Act.Sign, scale=-1.0)
                e = spool.tile([128, SD], BF16, tag="e")
                nc.scalar.activation(e[:, :], sg[:, :], Act.Relu)
            # t=0 has no flip (also guards Sign(0) semantics)
            nc.scalar.mul(
                e.rearrange("p (j ts) -> p j ts", ts=TS)[0:1, :, 0],
                e.rearrange("p (j ts) -> p j ts", ts=TS)[0:1, :, 0],
                0.0,
            )

            # within-octet inclusive prefix PARITY (segmented xor-scan):
            # state = (mask * state) xor e  -> 0/1 running parity per joint
            rowp = spool.tile([128, SD], FP32, tag="rowp")
            if probe == "noscan":
                nc.vector.tensor_copy(out=rowp[:, :], in_=e[:, :])
            else:
                eng_scan.tensor_tensor_scan(
                    out=rowp[:, :], data0=amask[:, :], data1=e[:, :],
                    initial=0.0, op0=Alu.mult, op1=Alu.logical_xor,
                )

            # octet-level: count of odd rows above (parity-sum via matmul)
            offs = offp.tile([128, J], FP32, tag="offs")
            nc.tensor.matmul(
                offs[:, :],
                lhsT=pmatSB[:, :],
                rhs=rowp.rearrange("p (j ts) -> p j ts", ts=TS)[:, :, 7],
                start=True,
                stop=True,
            )
            # parity of that count -> sigma_off in {+1, -1} per (p, j)
            sigo = spool.tile([128, J], BF16, tag="sigo")
            if sigo_cos:
                # count mod 2 on DVE (exact: counts are integer-valued fp32),
                # then the +-1 affine on ACT.
                offm = spool.tile([128, J], FP32, tag="offm")
                nc.vector.tensor_scalar(
                    out=offm[:, :], in0=offs[:, :], scalar1=2.0, scalar2=None,
                    op0=Alu.mod,
                )
                nc.scalar.activation(sigo[:, :], offm[:, :], Act.Copy,
                                     bias=1.0, scale=-2.0)
            else:
                offi = spool.tile([128, J], I32, tag="offi")
                nc.vector.tensor_copy(out=offi[:, :], in_=offs[:, :])
                offb = spool.tile([128, J], I32, tag="offb")
                nc.vector.tensor_scalar(
                    out=offb[:, :], in0=offi[:, :], scalar1=1, scalar2=None,
                    op0=Alu.bitwise_and,
                )
                nc.scalar.activation(sigo[:, :], offb[:, :], Act.Copy,
                                     bias=1.0, scale=-2.0)
            # sigma_row in {+1, -1} from the 0/1 row parity  (ACT)
            sigr = spool.tile([128, SD], BF16, tag="sigr")
            nc.scalar.activation(sigr[:, :], rowp[:, :], Act.Copy,
                                 bias=1.0, scale=-2.0)
            # sigma = sigma_row * sigma_off, (j, ts) layout
            sig = spool.tile([128, SD], BF16, tag="sig")
            eng_sig.tensor_tensor(
                out=sig.rearrange("p (j ts) -> p j ts", ts=TS),
                in0=sigr.rearrange("p (j ts) -> p j ts", ts=TS),
                in1=_ap(sigo, [[1, J], [0, TS]]),
                op=Alu.mult,
            )

            # out = q * sigma (broadcast over c), exact +/-1 multiply;
            # split by ts-range between VectorE and GpSimd.  With out_bf16
            # the result is written (rounded) to a bf16 tile and the store
            # moves half the bytes.
            if out_bf16:
                dst = bopool.tile([128, FD], BF16, tag="ob")
            else:
                dst = o
            if out_bf16 and cast_act:
                # bf16 copy of q on ACT so the final mult runs 16-bit on DVE
                qb = bopool.tile([128, FD], BF16, tag="qb")
                nc.scalar.copy(qb[:, :], qt[:, :])
                qsrc = qb
            else:
                qsrc = qt
            qv = qsrc.rearrange("p (ts x) -> p ts x", ts=TS)
            ow = dst.rearrange("p (ts x) -> p ts x", ts=TS)
            tsplit = mult_split
            if probe == "halfmult":
                tsplit = 4
                nc.vector.tensor_tensor(
                    out=ow[:, 0:4, :],
                    in0=qv[:, 0:4, :],
                    in1=bass.AP(
                        tensor=sig.tensor, offset=sig.offset,
                        ap=[list(sig.ap[0]), [1, 4], [TS, J], [0, C]],
                    ),
                    op=Alu.mult,
                )
            elif tsplit > 0:
                nc.vector.tensor_tensor(
                    out=ow[:, 0:tsplit, :],
                    in0=qv[:, 0:tsplit, :],
                    in1=bass.AP(
                        tensor=sig.tensor, offset=sig.offset,
                        ap=[list(sig.ap[0]), [1, tsplit], [TS, J], [0, C]],
                    ),
                    op=Alu.mult,
                )
            if tsplit < TS:
                nc.gpsimd.tensor_tensor(
                    out=ow[:, tsplit:TS, :],
                    in0=qv[:, tsplit:TS, :],
                    in1=bass.AP(
                        tensor=sig.tensor, offset=sig.offset + tsplit,
                        ap=[list(sig.ap[0]), [1, TS - tsplit], [TS, J],
                            [0, C]],
                    ),
                    op=Alu.mult,
                )

            eng_out.dma_start(
                out=of[b, :].rearrange("(p x) -> p x", p=128), in_=dst[:, :]
            )

        if unroll:
            for _ in range(reps):
                emit_body()
        elif reps == 1:
            emit_body()
        else:
            assert reps % loop_unroll == 0
            with tc.For_i(0, reps // loop_unroll, 1,
                          staggered_reset=bool(staggered)):
                for _ in range(loop_unroll):
                    emit_body()
    return nc


def make_consts():
    smat = np.eye(128, k=1, dtype=np.float32)       # S[k, m] = 1 iff m == k+1
    pmat = np.triu(np.ones((128, 128), np.float32), k=1)  # strict prefix
    return smat, pmat


def make_in_maps(q, smat, pmat):
    return [
        {"q": q[c * BPC:(c + 1) * BPC], "smat": smat, "pmat": pmat}
        for c in range(NCORES)
    ]


def kernel(joint_rotations: np.ndarray) -> np.ndarray:
    q = np.ascontiguousarray(joint_rotations, dtype=np.float32)
    assert q.shape == (B, T, J, C)
    smat, pmat = make_consts()
    nc = build_nc()
    nc.finalize()   # run bacc passes (wait splitting, reg alloc) + freeze
    in_maps = make_in_maps(q, smat, pmat)
    res = run_bass_kernel_spmd(nc, in_maps, list(range(NCORES)))
    outs = [np.asarray(r["out"]).astype(np.float32) for r in res.results]
    return np.concatenate(outs, axis=0)



# revision 23
# speedup vs baseline: 2.9611x; 1.0567x over previous
"""Trainium2 Bass kernel for BatchRemoveQuatDiscontinuities.

Algorithm (per (batch, joint) lane):
    d[t]    = dot(q[t], q[t-1])                (fp32, 4-wide dot)
    flip[t] = 1 if d[t] < 0 else 0             (t >= 1; flip[0] = 0)
    sigma[t] = (-1)^(sum_{s<=t} flip[s])       (cumulative sign parity)
    out[t]  = q[t] * sigma[t]

Mapping on a NeuronCore (data-parallel over batch across 8 cores):
  * One tile = one batch clip, loaded as a single fully-contiguous 1MB
    DMA: [128 partitions = t/8, free = (ts: 8, j: 64, c: 4)].  Loads on
    the SP HWDGE ring (nc.sync), stores on the ACT ring (nc.scalar).
  * q[t-1]: within a partition it is a free-axis offset (-256); the
    octet boundary (ts=0) needs q[p-1, ts=7], produced by a TensorE
    matmul with an off-diagonal 0/1 matrix S into PSUM (fp32 exact).
  * prod on DVE, 4-wide dot via two pairwise adds (c0+c1)+(c2+c3),
    d written in (j, ts) order; flip indicator e = Relu(Sign(-d)) on
    ScalarE (bf16).
  * Within-octet inclusive prefix: tensor_tensor_scan with a reset mask
    (state = mask*state xor e), segments of 8 per joint.  Octet-level
    parity via strict-triangular matmul over partitions on the per-row
    totals; parity of the count -> sigo (int &1, ACT affine to +-1).
  * sigma_row = 1-2*rowp on ScalarE (bf16); sig = sigr*sigo on GpSimd.
    Final out = q * sig (broadcast over c), ts-split DVE/GpSimd by
    mult_split - exact +/-1 multiply.

Engine budget (HW-measured, ablation-profiled): DVE is 100% critical at
~1.1-1.45ns/free-elem fp32 (prod 36us + u/d 36us + scan 11us + final
mult 36us per rep of 16 clips); 16-bit gives NO 2x here (broadcast
in1 AP blocks perf mode).  GpSimd TT hurts at ANY dose (ms7 = +4us) -
keep Pool idle.  ScalarE has ~60us slack and absorbs the bf16 cast.
DMA: load-only 45.6us/16MB (351GB/s), aggregate ~330GB/s/core; at 24MB
(fp32 in + bf16 out) DMA is NOT binding.  tensor_reduce(X) runs at
input rate (no win over pairwise adds); scalar_tensor_tensor is_gt is
~4x slower than plain TT (don't fuse e); tensor_tensor_scan does not
lower on Pool.  tc.For_i costs ~14us/iteration (all-engine barrier at
the back edge; staggered_reset no help) - amortize with loop_unroll.
Best config: out_bf16=1 cast_act=1 mult_split=8 sigm_xor=1 (body
~130us vs 165us for the old ms4 config, which was Pool-bound at 16K
elems ~ 10ns/elem).  sigm_xor: two bf16 outputs share one sigma, so the
final multiply is an int32 XOR of packed sign bits (0x80008000 mask) at
HALF the element count - TT fp32/broadcast never gets DVE 2x (only
2x_1p exists for TT and it needs all-packed 2-byte operands), so
shrinking element count is the only lever.
"""

import numpy as np
from contextlib import ExitStack

import concourse.bass as bass
import concourse.bacc as bacc
import concourse.tile as tile
from concourse import mybir
from concourse.bass_utils import run_bass_kernel_spmd

B, T, J, C = 128, 1024, 64, 4
NCORES = 8
JC = J * C                      # 256 floats per t
BPC = B // NCORES               # 16 batch clips per core
TS = 8                          # t per partition (octet)
FD = TS * JC                    # tile free dim = 2048 floats
SD = J * TS                     # prefix free dim = 512 (j, ts)

FP32 = mybir.dt.float32
BF16 = mybir.dt.bfloat16
I32 = mybir.dt.int32
Alu = mybir.AluOpType
Act = mybir.ActivationFunctionType


def _ap(apx, dims):
    """AP with explicit [step, count] free dims appended to partition dim."""
    return bass.AP(
        tensor=apx.tensor, offset=apx.offset,
        ap=[list(apx.ap[0]), *[list(d) for d in dims]],
    )


def build_nc(bpc=BPC, t=T, reps=1, mode="full", mult_split=8,
             sig_eng="dve", d_eng="dve", u_eng="dve", scan_eng="dve",
             out_ring="sync", qbufs=8, obufs=4, sbufs=4, sigo_cos=0,
             out_bf16=1, bobufs=5, cp=1, unroll=0, loop_unroll=1,
             staggered=0, fuse_e=0, e_eng="dve", cast_act=1, u_pool=0,
             probe="none", dred=0, sigm_xor=1):
    assert t % (128 * TS) == 0
    nc = bacc.Bacc(None, target_bir_lowering=False)
    q = nc.declare_dram_parameter("q", [bpc, t, J, C], FP32, isOutput=False)
    smat = nc.declare_dram_parameter("smat", [128, 128], FP32, isOutput=False)
    pmat = nc.declare_dram_parameter("pmat", [128, 128], FP32, isOutput=False)
    out_dt = BF16 if out_bf16 else FP32
    out = nc.declare_dram_parameter("out", [bpc, t, J, C], out_dt,
                                    isOutput=True)
    qf = q.rearrange("b t j c -> b (t j c)")
    of = out.rearrange("b t j c -> b (t j c)")

    eng_sig = nc.gpsimd if sig_eng == "pool" else nc.vector
    eng_d = nc.gpsimd if d_eng == "pool" else nc.vector
    eng_u = nc.gpsimd if u_eng == "pool" else nc.vector
    eng_scan = nc.gpsimd if scan_eng == "pool" else nc.vector
    eng_e = nc.gpsimd if e_eng == "pool" else nc.vector
    eng_out = nc.scalar if out_ring == "act" else nc.sync

    with tile.TileContext(nc) as tc, ExitStack() as ctx:
        consts = ctx.enter_context(tc.tile_pool(name="consts", bufs=1))
        qpool = ctx.enter_context(tc.tile_pool(name="qpool", bufs=qbufs))
        opool = ctx.enter_context(tc.tile_pool(name="opool", bufs=obufs))
        bopool = (ctx.enter_context(tc.tile_pool(name="bopool", bufs=bobufs))
                  if out_bf16 else None)
        spool = ctx.enter_context(tc.tile_pool(name="spool", bufs=sbufs))
        auxp = ctx.enter_context(tc.tile_pool(name="auxp", bufs=4, space="PSUM"))
        offp = ctx.enter_context(tc.tile_pool(name="offp", bufs=4, space="PSUM"))

        smatSB = consts.tile([128, 128], FP32)
        nc.sync.dma_start(out=smatSB[:, :], in_=smat[:, :])
        pmatSB = consts.tile([128, 128], FP32)
        nc.sync.dma_start(out=pmatSB[:, :], in_=pmat[:, :])
        amask = consts.tile([128, cp * SD], FP32)
        nc.vector.memset(amask[:, :], 1.0)
        nc.vector.memset(
            amask.rearrange("p (b j ts) -> p b j ts", b=cp, ts=TS)[:, :, :, 0],
            0.0,
        )
        pihalf = consts.tile([128, 1], FP32)
        nc.vector.memset(pihalf[:, :], float(np.pi / 2))
        msk32 = consts.tile([128, 1], I32)
        # sign bits of a packed bf16 pair: 0x80008000 as int32
        nc.vector.memset(msk32[:, :], -2147450880)

        def emit_body():
            if cp == 1:
                for b in range(bpc):
                    emit_tile(b)
            else:
                for g in range(bpc // cp):
                    emit_tile_cp2(g)

        def emit_tile_cp2(g):
            """Two clips per tile: same per-engine work doses as the cp=1
            path (Pool only gets the final-mult halves), but the 1024-wide
            stages (prod/scan/sig/ACT chain) are fused across the clip pair
            to halve instruction counts, and the DMAs are 2MB."""
            qt = qpool.tile([128, cp, FD], FP32, tag="qt")
            nc.sync.dma_start(
                out=qt[:, :, :],
                in_=qf[g * cp:(g + 1) * cp, :].rearrange(
                    "b (p x) -> p b x", p=128),
            )
            o = opool.tile([128, cp, FD], FP32, tag="o")
            aux = auxp.tile([128, cp, JC], FP32, tag="aux")
            nc.tensor.matmul(
                aux[:, :, :], lhsT=smatSB[:, :], rhs=qt[:, :, FD - JC:FD],
                start=True, stop=True,
            )
            nc.vector.tensor_tensor(
                out=o[:, :, JC:FD], in0=qt[:, :, JC:FD],
                in1=qt[:, :, 0:FD - JC], op=Alu.mult,
            )
            nc.vector.tensor_tensor(
                out=o[:, :, 0:JC], in0=qt[:, :, 0:JC], in1=aux[:, :, :],
                op=Alu.mult,
            )
            u = spool.tile([128, cp, 2 * SD], FP32, tag="u")
            d = spool.tile([128, cp, SD], FP32, tag="d")
            for c2 in range(cp):
                opairs = o.rearrange(
                    "p b (s k two) -> p b s k two", k=2, two=2)[:, c2]
                uv = u.rearrange("p b (s k) -> p b s k", k=2)[:, c2]
                nc.vector.tensor_tensor(
                    out=uv, in0=opairs[:, :, :, 0], in1=opairs[:, :, :, 1],
                    op=Alu.add,
                )
                u_k = u.rearrange(
                    "p b (ts j k) -> p b ts j k", j=J, k=2)[:, c2]
                dv = bass.AP(
                    tensor=d.tensor, offset=d.offset + c2 * SD,
                    ap=[list(d.ap[0]), [1, TS], [TS, J]],
                )
                nc.vector.tensor_tensor(
                    out=dv, in0=u_k[:, :, :, 0], in1=u_k[:, :, :, 1],
                    op=Alu.add,
                )
            df = d.rearrange("p b s -> p (b s)")
            sg = spool.tile([128, cp * SD], FP32, tag="sg")
            nc.scalar.activation(sg[:, :], df, Act.Sign, scale=-1.0)
            e = spool.tile([128, cp * SD], BF16, tag="e")
            nc.scalar.activation(e[:, :], sg[:, :], Act.Relu)
            ev = e.rearrange("p (b j ts) -> p b j ts", b=cp, ts=TS)
            nc.scalar.mul(ev[0:1, :, :, 0], ev[0:1, :, :, 0], 0.0)
            rowp = spool.tile([128, cp * SD], FP32, tag="rowp")
            nc.vector.tensor_tensor_scan(
                out=rowp[:, :], data0=amask[:, :], data1=e[:, :],
                initial=0.0, op0=Alu.mult, op1=Alu.logical_xor,
            )
            offs = offp.tile([128, cp, J], FP32, tag="offs")
            rr = rowp.rearrange("p (b j ts) -> p b j ts", b=cp, ts=TS)
            nc.tensor.matmul(
                offs[:, :, :], lhsT=pmatSB[:, :], rhs=rr[:, :, :, 7],
                start=True, stop=True,
            )
            offi = spool.tile([128, cp * J], I32, tag="offi")
            nc.vector.tensor_copy(
                out=offi[:, :], in_=offs.rearrange("p b j -> p (b j)"))
            offb = spool.tile([128, cp * J], I32, tag="offb")
            nc.vector.tensor_scalar(
                out=offb[:, :], in0=offi[:, :], scalar1=1, scalar2=None,
                op0=Alu.bitwise_and,
            )
            sigo = spool.tile([128, cp * J], BF16, tag="sigo")
            nc.scalar.activation(sigo[:, :], offb[:, :], Act.Copy,
                                 bias=1.0, scale=-2.0)
            sigr = spool.tile([128, cp * SD], BF16, tag="sigr")
            nc.scalar.activation(sigr[:, :], rowp[:, :], Act.Copy,
                                 bias=1.0, scale=-2.0)
            sig = spool.tile([128, cp * SD], BF16, tag="sig")
            nc.vector.tensor_tensor(
                out=sig.rearrange("p (bj ts) -> p bj ts", ts=TS),
                in0=sigr.rearrange("p (bj ts) -> p bj ts", ts=TS),
                in1=_ap(sigo, [[1, cp * J], [0, TS]]),
                op=Alu.mult,
            )
            if out_bf16:
                dst = bopool.tile([128, cp, FD], BF16, tag="ob")
            else:
                dst = o
            if out_bf16 and cast_act:
                qb = bopool.tile([128, cp, FD], BF16, tag="qb")
                nc.scalar.copy(qb[:, :, :], qt[:, :, :])
                qsrc = qb
            else:
                qsrc = qt
            for c2 in range(cp):
                qv = qsrc.rearrange("p b (ts x) -> p b ts x", ts=TS)[:, c2]
                ow = dst.rearrange("p b (ts x) -> p b ts x", ts=TS)[:, c2]
                sbase = sig.offset + c2 * SD
                if mult_split > 0:
                    nc.vector.tensor_tensor(
                        out=ow[:, 0:mult_split, :],
                        in0=qv[:, 0:mult_split, :],
                        in1=bass.AP(
                            tensor=sig.tensor, offset=sbase,
                            ap=[list(sig.ap[0]), [1, mult_split], [TS, J],
                                [0, C]],
                        ),
                        op=Alu.mult,
                    )
                if mult_split < TS:
                    nc.gpsimd.tensor_tensor(
                        out=ow[:, mult_split:TS, :],
                        in0=qv[:, mult_split:TS, :],
                        in1=bass.AP(
                            tensor=sig.tensor, offset=sbase + mult_split,
                            ap=[list(sig.ap[0]), [1, TS - mult_split],
                                [TS, J], [0, C]],
                        ),
                        op=Alu.mult,
                    )
            eng_out.dma_start(
                out=of[g * cp:(g + 1) * cp, :].rearrange(
                    "b (p x) -> p b x", p=128),
                in_=dst[:, :, :],
            )

        stile = None
        if mode == "store":
            stile = consts.tile([128, FD], FP32)
            nc.vector.memset(stile[:, :], 1.0)

        def emit_tile(b):
            if mode == "store":
                eng_out.dma_start(
                    out=of[b, :].rearrange("(p x) -> p x", p=128),
                    in_=stile[:, :],
                )
                return
            qt = qpool.tile([128, FD], FP32, tag="qt")
            nc.sync.dma_start(
                out=qt[:, :],
                in_=qf[b, :].rearrange("(p x) -> p x", p=128),
            )
            if mode == "load":
                return
            o = opool.tile([128, FD], FP32, tag="o")
            if mode == "dma":
                eng_out.dma_start(
                    out=of[b, :].rearrange("(p x) -> p x", p=128), in_=qt[:, :]
                )
                return

            # octet-boundary shift: aux[p] = qt[p-1, ts=7 chunk] (row 0 = 0)
            aux = auxp.tile([128, JC], FP32, tag="aux")
            nc.tensor.matmul(
                aux[:, :],
                lhsT=smatSB[:, :],
                rhs=qt[:, FD - JC:FD],
                start=True,
                stop=True,
            )

            # prod: o = q * q_shifted  (DVE)
            if probe != "noprod":
                nc.vector.tensor_tensor(
                    out=o[:, JC:FD], in0=qt[:, JC:FD], in1=qt[:, 0:FD - JC],
                    op=Alu.mult,
                )
                nc.vector.tensor_tensor(
                    out=o[:, 0:JC], in0=qt[:, 0:JC], in1=aux[:, :],
                    op=Alu.mult,
                )
                osrc = o
            else:
                osrc = qt

            if dred:
                # d = reduce over c in one DVE pass, written in (j, ts) order
                d = spool.tile([128, SD], FP32, tag="d")
                nc.vector.tensor_reduce(
                    out=_ap(d, [[1, TS], [TS, J]]),
                    in_=osrc.rearrange("p (ts j c) -> p ts j c", j=J, c=C),
                    axis=mybir.AxisListType.X, op=Alu.add,
                )
                sg = spool.tile([128, SD], FP32, tag="sg")
                nc.scalar.activation(sg[:, :], d[:, :], Act.Sign, scale=-1.0)
                e = spool.tile([128, SD], BF16, tag="e")
                nc.scalar.activation(e[:, :], sg[:, :], Act.Relu)
            if not dred:
                # dot over c, pairwise (c0+c1)+(c2+c3); d in (j, ts) order
                u = spool.tile([128, 2 * SD], FP32, tag="u")
                ov = osrc.rearrange("p (s c) -> p s c", c=C)
                uv = u.rearrange("p (s k) -> p s k", k=2)
                opairs = ov.rearrange("p s (k two) -> p s k two", k=2)
                if probe == "noud":
                    pass
                elif u_pool:
                    nc.gpsimd.tensor_tensor(
                        out=uv[:, 0:u_pool], in0=opairs[:, 0:u_pool, :, 0],
                        in1=opairs[:, 0:u_pool, :, 1], op=Alu.add,
                    )
                    eng_u.tensor_tensor(
                        out=uv[:, u_pool:SD], in0=opairs[:, u_pool:SD, :, 0],
                        in1=opairs[:, u_pool:SD, :, 1], op=Alu.add,
                    )
                else:
                    eng_u.tensor_tensor(
                        out=uv, in0=opairs[:, :, :, 0], in1=opairs[:, :, :, 1],
                        op=Alu.add,
                    )
                u_k = u.rearrange("p (ts j k) -> p ts j k", j=J, k=2)
            if dred:
                pass
            elif probe == "noud":
                d = spool.tile([128, SD], FP32, tag="d")
                sg = spool.tile([128, SD], FP32, tag="sg")
                nc.scalar.activation(sg[:, :], amask[:, 0:SD], Act.Sign,
                                     scale=-1.0)
                e = spool.tile([128, SD], BF16, tag="e")
                nc.scalar.activation(e[:, :], sg[:, :], Act.Relu)
            elif fuse_e:
                # e = 1.0 iff (-u0 > u1) <=> d = u0 + u1 < 0, one DVE/Pool
                # op replacing the d add + ACT Sign + ACT Relu chain.  The
                # comparison of the two fp values is exact, and for near-tie
                # sums fl(u0+u1) is exact (Sterbenz), so this matches the
                # old fl(d) < 0 test bit-for-bit.
                e = spool.tile([128, SD], BF16, tag="e")
                eng_e.scalar_tensor_tensor(
                    out=_ap(e, [[1, TS], [TS, J]]),
                    in0=u_k[:, :, :, 0], scalar=-1.0, in1=u_k[:, :, :, 1],
                    op0=Alu.mult, op1=Alu.is_gt,
                )
            else:
                d = spool.tile([128, SD], FP32, tag="d")  # (j, ts) layout
                eng_d.tensor_tensor(
                    out=_ap(d, [[1, TS], [TS, J]]),
                    in0=u_k[:, :, :, 0],
                    in1=u_k[:, :, :, 1],
                    op=Alu.add,
                )

                # flip indicator e = Relu(Sign(-d)), bf16, (j, ts) layout
                sg = spool.tile([128, SD], FP32, tag="sg")
                nc.scalar.activation(sg[:, :], d[:, :], Act.Sign, scale=-1.0)
                e = spool.tile([128, SD], BF16, tag="e")
                nc.scalar.activation(e[:, :], sg[:, :], Act.Relu)
            # t=0 has no flip (also guards Sign(0) semantics)
            nc.scalar.mul(
                e.rearrange("p (j ts) -> p j ts", ts=TS)[0:1, :, 0],
                e.rearrange("p (j ts) -> p j ts", ts=TS)[0:1, :, 0],
                0.0,
            )

            # within-octet inclusive prefix PARITY (segmented xor-scan):
            # state = (mask * state) xor e  -> 0/1 running parity per joint
            rowp = spool.tile([128, SD], FP32, tag="rowp")
            if probe == "noscan":
                nc.vector.tensor_copy(out=rowp[:, :], in_=e[:, :])
            else:
                eng_scan.tensor_tensor_scan(
                    out=rowp[:, :], data0=amask[:, :], data1=e[:, :],
                    initial=0.0, op0=Alu.mult, op1=Alu.logical_xor,
                )

            # octet-level: count of odd rows above (parity-sum via matmul)
            offs = offp.tile([128, J], FP32, tag="offs")
            nc.tensor.matmul(
                offs[:, :],
                lhsT=pmatSB[:, :],
                rhs=rowp.rearrange("p (j ts) -> p j ts", ts=TS)[:, :, 7],
                start=True,
                stop=True,
            )
            if sigm_xor:
                # pre = rowp XOR oddbit (0/1 int32, (j, ts) layout); then
                # sigm = pre * 0x80008000 = sign-bit mask for a packed
                # bf16 pair.  Final mult becomes an int32 XOR at HALF the
                # element count (two bf16 share one sigma).
                offi = spool.tile([128, J], I32, tag="offi")
                nc.vector.tensor_copy(out=offi[:, :], in_=offs[:, :])
                offb = spool.tile([128, J], I32, tag="offb")
                nc.vector.tensor_scalar(
                    out=offb[:, :], in0=offi[:, :], scalar1=1, scalar2=None,
                    op0=Alu.bitwise_and,
                )
                pre = spool.tile([128, SD], I32, tag="pre")
                nc.vector.tensor_tensor(
                    out=pre.rearrange("p (j ts) -> p j ts", ts=TS),
                    in0=rowp.rearrange("p (j ts) -> p j ts", ts=TS),
                    in1=_ap(offb, [[1, J], [0, TS]]),
                    op=Alu.logical_xor,
                )
                sigm = spool.tile([128, SD], I32, tag="sigm")
                nc.vector.tensor_tensor(
                    out=sigm[:, :], in0=pre[:, :],
                    in1=_ap(msk32, [[0, SD]]), op=Alu.mult,
                )
                qb = bopool.tile([128, FD], BF16, tag="qb")
                nc.scalar.copy(qb[:, :], qt[:, :])
                dst = bopool.tile([128, FD], BF16, tag="ob")
                qb3 = qb[:, :].bitcast(I32).rearrange(
                    "p (ts j cc) -> p ts j cc", ts=TS, j=J)
                ob3 = dst[:, :].bitcast(I32).rearrange(
                    "p (ts j cc) -> p ts j cc", ts=TS, j=J)
                nc.vector.tensor_tensor(
                    out=ob3, in0=qb3,
                    in1=bass.AP(
                        tensor=sigm.tensor, offset=sigm.offset,
                        ap=[list(sigm.ap[0]), [1, TS], [TS, J], [0, 2]],
                    ),
                    op=Alu.bitwise_xor,
                )
                eng_out.dma_start(
                    out=of[b, :].rearrange("(p x) -> p x", p=128),
                    in_=dst[:, :],
                )
                return

            # parity of that count -> sigma_off in {+1, -1} per (p, j)
            sigo = spool.tile([128, J], BF16, tag="sigo")
            if sigo_cos:
                # count mod 2 on DVE (exact: counts are integer-valued fp32),
                # then the +-1 affine on ACT.
                offm = spool.tile([128, J], FP32, tag="offm")
                nc.vector.tensor_scalar(
                    out=offm[:, :], in0=offs[:, :], scalar1=2.0, scalar2=None,
                    op0=Alu.mod,
                )
                nc.scalar.activation(sigo[:, :], offm[:, :], Act.Copy,
                                     bias=1.0, scale=-2.0)
            else:
                offi = spool.tile([128, J], I32, tag="offi")
                nc.vector.tensor_copy(out=offi[:, :], in_=offs[:, :])
                offb = spool.tile([128, J], I32, tag="offb")
                nc.vector.tensor_scalar(
                    out=offb[:, :], in0=offi[:, :], scalar1=1, scalar2=None,
                    op0=Alu.bitwise_and,
                )
                nc.scalar.activation(sigo[:, :], offb[:, :], Act.Copy,
                                     bias=1.0, scale=-2.0)
            # sigma_row in {+1, -1} from the 0/1 row parity  (ACT)
            sigr = spool.tile([128, SD], BF16, tag="sigr")
            nc.scalar.activation(sigr[:, :], rowp[:, :], Act.Copy,
                                 bias=1.0, scale=-2.0)
            # sigma = sigma_row * sigma_off, (j, ts) layout
            sig = spool.tile([128, SD], BF16, tag="sig")
            eng_sig.tensor_tensor(
                out=sig.rearrange("p (j ts) -> p j ts", ts=TS),
                in0=sigr.rearrange("p (j ts) -> p j ts", ts=TS),
                in1=_ap(sigo, [[1, J], [0, TS]]),
                op=Alu.mult,
            )

            # out = q * sigma (broadcast over c), exact +/-1 multiply;
            # split by ts-range between VectorE and GpSimd.  With out_bf16
            # the result is written (rounded) to a bf16 tile and the store
            # moves half the bytes.
            if out_bf16:
                dst = bopool.tile([128, FD], BF16, tag="ob")
            else:
                dst = o
            if out_bf16 and cast_act:
                # bf16 copy of q on ACT so the final mult runs 16-bit on DVE
                qb = bopool.tile([128, FD], BF16, tag="qb")
                nc.scalar.copy(qb[:, :], qt[:, :])
                qsrc = qb
            else:
                qsrc = qt
            qv = qsrc.rearrange("p (ts x) -> p ts x", ts=TS)
            ow = dst.rearrange("p (ts x) -> p ts x", ts=TS)
            tsplit = mult_split
            if probe == "halfmult":
                tsplit = 4
                nc.vector.tensor_tensor(
                    out=ow[:, 0:4, :],
                    in0=qv[:, 0:4, :],
                    in1=bass.AP(
                        tensor=sig.tensor, offset=sig.offset,
                        ap=[list(sig.ap[0]), [1, 4], [TS, J], [0, C]],
                    ),
                    op=Alu.mult,
                )
            elif tsplit > 0:
                nc.vector.tensor_tensor(
                    out=ow[:, 0:tsplit, :],
                    in0=qv[:, 0:tsplit, :],
                    in1=bass.AP(
                        tensor=sig.tensor, offset=sig.offset,
                        ap=[list(sig.ap[0]), [1, tsplit], [TS, J], [0, C]],
                    ),
                    op=Alu.mult,
                )
            if tsplit < TS:
                nc.gpsimd.tensor_tensor(
                    out=ow[:, tsplit:TS, :],
                    in0=qv[:, tsplit:TS, :],
                    in1=bass.AP(
                        tensor=sig.tensor, offset=sig.offset + tsplit,
                        ap=[list(sig.ap[0]), [1, TS - tsplit], [TS, J],
                            [0, C]],
                    ),
                    op=Alu.mult,
                )

            eng_out.dma_start(
                out=of[b, :].rearrange("(p x) -> p x", p=128), in_=dst[:, :]
            )

        if unroll:
            for _ in range(reps):
                emit_body()
        elif reps == 1:
            emit_body()
        else:
            assert reps % loop_unroll == 0
            with tc.For_i(0, reps // loop_unroll, 1,
                          staggered_reset=bool(staggered)):
                for _ in range(loop_unroll):
                    emit_body()
    return nc


def make_consts():
    smat = np.eye(128, k=1, dtype=np.float32)       # S[k, m] = 1 iff m == k+1
    pmat = np.triu(np.ones((128, 128), np.float32), k=1)  # strict prefix
    return smat, pmat


def make_in_maps(q, smat, pmat):
    return [
        {"q": q[c * BPC:(c + 1) * BPC], "smat": smat, "pmat": pmat}
        for c in range(NCORES)
    ]


def kernel(joint_rotations: np.ndarray) -> np.ndarray:
    q = np.ascontiguousarray(joint_rotations, dtype=np.float32)
    assert q.shape == (B, T, J, C)
    smat, pmat = make_consts()
    nc = build_nc()
    nc.finalize()   # run bacc passes (wait splitting, reg alloc) + freeze
    in_maps = make_in_maps(q, smat, pmat)
    res = run_bass_kernel_spmd(nc, in_maps, list(range(NCORES)))
    outs = [np.asarray(r["out"]).astype(np.float32) for r in res.results]
    return np.concatenate(outs, axis=0)



# revision 24
# speedup vs baseline: 2.9656x; 1.0015x over previous
"""Trainium2 Bass kernel for BatchRemoveQuatDiscontinuities.

Algorithm (per (batch, joint) lane):
    d[t]    = dot(q[t], q[t-1])                (fp32, 4-wide dot)
    flip[t] = 1 if d[t] < 0 else 0             (t >= 1; flip[0] = 0)
    sigma[t] = (-1)^(sum_{s<=t} flip[s])       (cumulative sign parity)
    out[t]  = q[t] * sigma[t]

Mapping on a NeuronCore (data-parallel over batch across 8 cores):
  * One tile = one batch clip, loaded as a single fully-contiguous 1MB
    DMA: [128 partitions = t/8, free = (ts: 8, j: 64, c: 4)].  Loads on
    the SP HWDGE ring (nc.sync), stores on the ACT ring (nc.scalar).
  * q[t-1]: within a partition it is a free-axis offset (-256); the
    octet boundary (ts=0) needs q[p-1, ts=7], produced by a TensorE
    matmul with an off-diagonal 0/1 matrix S into PSUM (fp32 exact).
  * prod on DVE, 4-wide dot via two pairwise adds (c0+c1)+(c2+c3),
    d written in (j, ts) order; flip indicator e = Relu(Sign(-d)) on
    ScalarE (bf16).
  * Within-octet inclusive prefix: tensor_tensor_scan with a reset mask
    (state = mask*state xor e), segments of 8 per joint.  Octet-level
    parity via strict-triangular matmul over partitions on the per-row
    totals; parity of the count -> sigo (int &1, ACT affine to +-1).
  * sigma_row = 1-2*rowp on ScalarE (bf16); sig = sigr*sigo on GpSimd.
    Final out = q * sig (broadcast over c), ts-split DVE/GpSimd by
    mult_split - exact +/-1 multiply.

Engine budget (HW-measured, ablation-profiled): DVE is 100% critical at
~1.1-1.45ns/free-elem fp32 (prod 36us + u/d 36us + scan 11us + final
mult 36us per rep of 16 clips); 16-bit gives NO 2x here (broadcast
in1 AP blocks perf mode).  GpSimd TT hurts at ANY dose (ms7 = +4us) -
keep Pool idle.  ScalarE has ~60us slack and absorbs the bf16 cast.
DMA: load-only 45.6us/16MB (351GB/s), aggregate ~330GB/s/core; at 24MB
(fp32 in + bf16 out) DMA is NOT binding.  tensor_reduce(X) runs at
input rate (no win over pairwise adds); scalar_tensor_tensor is_gt is
~4x slower than plain TT (don't fuse e); tensor_tensor_scan does not
lower on Pool.  tc.For_i costs ~14us/iteration (all-engine barrier at
the back edge; staggered_reset no help) - amortize with loop_unroll.
Best config: out_bf16=1 cast_act=1 mult_split=8 sigm_xor=1 (body
~130us vs 165us for the old ms4 config, which was Pool-bound at 16K
elems ~ 10ns/elem).  sigm_xor: two bf16 outputs share one sigma, so the
final multiply is an int32 XOR of packed sign bits (0x80008000 mask) at
HALF the element count - TT fp32/broadcast never gets DVE 2x (only
2x_1p exists for TT and it needs all-packed 2-byte operands), so
shrinking element count is the only lever.
"""

import numpy as np
from contextlib import ExitStack

import concourse.bass as bass
import concourse.bacc as bacc
import concourse.tile as tile
from concourse import mybir
from concourse.bass_utils import run_bass_kernel_spmd

B, T, J, C = 128, 1024, 64, 4
NCORES = 8
JC = J * C                      # 256 floats per t
BPC = B // NCORES               # 16 batch clips per core
TS = 8                          # t per partition (octet)
FD = TS * JC                    # tile free dim = 2048 floats
SD = J * TS                     # prefix free dim = 512 (j, ts)

FP32 = mybir.dt.float32
BF16 = mybir.dt.bfloat16
I32 = mybir.dt.int32
Alu = mybir.AluOpType
Act = mybir.ActivationFunctionType


def _ap(apx, dims):
    """AP with explicit [step, count] free dims appended to partition dim."""
    return bass.AP(
        tensor=apx.tensor, offset=apx.offset,
        ap=[list(apx.ap[0]), *[list(d) for d in dims]],
    )


def build_nc(bpc=BPC, t=T, reps=1, mode="full", mult_split=8,
             sig_eng="dve", d_eng="dve", u_eng="dve", scan_eng="dve",
             out_ring="sync", qbufs=8, obufs=4, sbufs=4, sigo_cos=0,
             out_bf16=1, bobufs=5, cp=1, unroll=0, loop_unroll=1,
             staggered=0, fuse_e=0, e_eng="dve", cast_act=1, u_pool=0,
             probe="none", dred=0, sigm_xor=1, u2=0):
    assert t % (128 * TS) == 0
    nc = bacc.Bacc(None, target_bir_lowering=False)
    q = nc.declare_dram_parameter("q", [bpc, t, J, C], FP32, isOutput=False)
    smat = nc.declare_dram_parameter("smat", [128, 128], FP32, isOutput=False)
    pmat = nc.declare_dram_parameter("pmat", [128, 128], FP32, isOutput=False)
    out_dt = BF16 if out_bf16 else FP32
    out = nc.declare_dram_parameter("out", [bpc, t, J, C], out_dt,
                                    isOutput=True)
    qf = q.rearrange("b t j c -> b (t j c)")
    of = out.rearrange("b t j c -> b (t j c)")

    eng_sig = nc.gpsimd if sig_eng == "pool" else nc.vector
    eng_d = nc.gpsimd if d_eng == "pool" else nc.vector
    eng_u = nc.gpsimd if u_eng == "pool" else nc.vector
    eng_scan = nc.gpsimd if scan_eng == "pool" else nc.vector
    eng_e = nc.gpsimd if e_eng == "pool" else nc.vector
    eng_out = nc.scalar if out_ring == "act" else nc.sync

    with tile.TileContext(nc) as tc, ExitStack() as ctx:
        consts = ctx.enter_context(tc.tile_pool(name="consts", bufs=1))
        qpool = ctx.enter_context(tc.tile_pool(name="qpool", bufs=qbufs))
        opool = ctx.enter_context(tc.tile_pool(name="opool", bufs=obufs))
        bopool = (ctx.enter_context(tc.tile_pool(name="bopool", bufs=bobufs))
                  if out_bf16 else None)
        spool = ctx.enter_context(tc.tile_pool(name="spool", bufs=sbufs))
        auxp = ctx.enter_context(tc.tile_pool(name="auxp", bufs=4, space="PSUM"))
        offp = ctx.enter_context(tc.tile_pool(name="offp", bufs=4, space="PSUM"))

        smatSB = consts.tile([128, 128], FP32)
        nc.sync.dma_start(out=smatSB[:, :], in_=smat[:, :])
        pmatSB = consts.tile([128, 128], FP32)
        nc.sync.dma_start(out=pmatSB[:, :], in_=pmat[:, :])
        amask = consts.tile([128, cp * SD], FP32)
        nc.vector.memset(amask[:, :], 1.0)
        nc.vector.memset(
            amask.rearrange("p (b j ts) -> p b j ts", b=cp, ts=TS)[:, :, :, 0],
            0.0,
        )
        pihalf = consts.tile([128, 1], FP32)
        nc.vector.memset(pihalf[:, :], float(np.pi / 2))
        msk32 = consts.tile([128, 1], I32)
        # sign bits of a packed bf16 pair: 0x80008000 as int32
        nc.vector.memset(msk32[:, :], -2147450880)

        def emit_body():
            if cp == 1:
                for b in range(bpc):
                    emit_tile(b)
            else:
                for g in range(bpc // cp):
                    emit_tile_cp2(g)

        def emit_tile_cp2(g):
            """Two clips per tile: same per-engine work doses as the cp=1
            path (Pool only gets the final-mult halves), but the 1024-wide
            stages (prod/scan/sig/ACT chain) are fused across the clip pair
            to halve instruction counts, and the DMAs are 2MB."""
            qt = qpool.tile([128, cp, FD], FP32, tag="qt")
            nc.sync.dma_start(
                out=qt[:, :, :],
                in_=qf[g * cp:(g + 1) * cp, :].rearrange(
                    "b (p x) -> p b x", p=128),
            )
            o = opool.tile([128, cp, FD], FP32, tag="o")
            aux = auxp.tile([128, cp, JC], FP32, tag="aux")
            nc.tensor.matmul(
                aux[:, :, :], lhsT=smatSB[:, :], rhs=qt[:, :, FD - JC:FD],
                start=True, stop=True,
            )
            nc.vector.tensor_tensor(
                out=o[:, :, JC:FD], in0=qt[:, :, JC:FD],
                in1=qt[:, :, 0:FD - JC], op=Alu.mult,
            )
            nc.vector.tensor_tensor(
                out=o[:, :, 0:JC], in0=qt[:, :, 0:JC], in1=aux[:, :, :],
                op=Alu.mult,
            )
            u = spool.tile([128, cp, 2 * SD], FP32, tag="u")
            d = spool.tile([128, cp, SD], FP32, tag="d")
            for c2 in range(cp):
                opairs = o.rearrange(
                    "p b (s k two) -> p b s k two", k=2, two=2)[:, c2]
                uv = u.rearrange("p b (s k) -> p b s k", k=2)[:, c2]
                nc.vector.tensor_tensor(
                    out=uv, in0=opairs[:, :, :, 0], in1=opairs[:, :, :, 1],
                    op=Alu.add,
                )
                u_k = u.rearrange(
                    "p b (ts j k) -> p b ts j k", j=J, k=2)[:, c2]
                dv = bass.AP(
                    tensor=d.tensor, offset=d.offset + c2 * SD,
                    ap=[list(d.ap[0]), [1, TS], [TS, J]],
                )
                nc.vector.tensor_tensor(
                    out=dv, in0=u_k[:, :, :, 0], in1=u_k[:, :, :, 1],
                    op=Alu.add,
                )
            df = d.rearrange("p b s -> p (b s)")
            sg = spool.tile([128, cp * SD], FP32, tag="sg")
            nc.scalar.activation(sg[:, :], df, Act.Sign, scale=-1.0)
            e = spool.tile([128, cp * SD], BF16, tag="e")
            nc.scalar.activation(e[:, :], sg[:, :], Act.Relu)
            ev = e.rearrange("p (b j ts) -> p b j ts", b=cp, ts=TS)
            nc.scalar.mul(ev[0:1, :, :, 0], ev[0:1, :, :, 0], 0.0)
            rowp = spool.tile([128, cp * SD], FP32, tag="rowp")
            nc.vector.tensor_tensor_scan(
                out=rowp[:, :], data0=amask[:, :], data1=e[:, :],
                initial=0.0, op0=Alu.mult, op1=Alu.logical_xor,
            )
            offs = offp.tile([128, cp, J], FP32, tag="offs")
            rr = rowp.rearrange("p (b j ts) -> p b j ts", b=cp, ts=TS)
            nc.tensor.matmul(
                offs[:, :, :], lhsT=pmatSB[:, :], rhs=rr[:, :, :, 7],
                start=True, stop=True,
            )
            offi = spool.tile([128, cp * J], I32, tag="offi")
            nc.vector.tensor_copy(
                out=offi[:, :], in_=offs.rearrange("p b j -> p (b j)"))
            offb = spool.tile([128, cp * J], I32, tag="offb")
            nc.vector.tensor_scalar(
                out=offb[:, :], in0=offi[:, :], scalar1=1, scalar2=None,
                op0=Alu.bitwise_and,
            )
            sigo = spool.tile([128, cp * J], BF16, tag="sigo")
            nc.scalar.activation(sigo[:, :], offb[:, :], Act.Copy,
                                 bias=1.0, scale=-2.0)
            sigr = spool.tile([128, cp * SD], BF16, tag="sigr")
            nc.scalar.activation(sigr[:, :], rowp[:, :], Act.Copy,
                                 bias=1.0, scale=-2.0)
            sig = spool.tile([128, cp * SD], BF16, tag="sig")
            nc.vector.tensor_tensor(
                out=sig.rearrange("p (bj ts) -> p bj ts", ts=TS),
                in0=sigr.rearrange("p (bj ts) -> p bj ts", ts=TS),
                in1=_ap(sigo, [[1, cp * J], [0, TS]]),
                op=Alu.mult,
            )
            if out_bf16:
                dst = bopool.tile([128, cp, FD], BF16, tag="ob")
            else:
                dst = o
            if out_bf16 and cast_act:
                qb = bopool.tile([128, cp, FD], BF16, tag="qb")
                nc.scalar.copy(qb[:, :, :], qt[:, :, :])
                qsrc = qb
            else:
                qsrc = qt
            for c2 in range(cp):
                qv = qsrc.rearrange("p b (ts x) -> p b ts x", ts=TS)[:, c2]
                ow = dst.rearrange("p b (ts x) -> p b ts x", ts=TS)[:, c2]
                sbase = sig.offset + c2 * SD
                if mult_split > 0:
                    nc.vector.tensor_tensor(
                        out=ow[:, 0:mult_split, :],
                        in0=qv[:, 0:mult_split, :],
                        in1=bass.AP(
                            tensor=sig.tensor, offset=sbase,
                            ap=[list(sig.ap[0]), [1, mult_split], [TS, J],
                                [0, C]],
                        ),
                        op=Alu.mult,
                    )
                if mult_split < TS:
                    nc.gpsimd.tensor_tensor(
                        out=ow[:, mult_split:TS, :],
                        in0=qv[:, mult_split:TS, :],
                        in1=bass.AP(
                            tensor=sig.tensor, offset=sbase + mult_split,
                            ap=[list(sig.ap[0]), [1, TS - mult_split],
                                [TS, J], [0, C]],
                        ),
                        op=Alu.mult,
                    )
            eng_out.dma_start(
                out=of[g * cp:(g + 1) * cp, :].rearrange(
                    "b (p x) -> p b x", p=128),
                in_=dst[:, :, :],
            )

        stile = None
        if mode == "store":
            stile = consts.tile([128, FD], FP32)
            nc.vector.memset(stile[:, :], 1.0)

        def emit_tile(b):
            if mode == "store":
                eng_out.dma_start(
                    out=of[b, :].rearrange("(p x) -> p x", p=128),
                    in_=stile[:, :],
                )
                return
            qt = qpool.tile([128, FD], FP32, tag="qt")
            nc.sync.dma_start(
                out=qt[:, :],
                in_=qf[b, :].rearrange("(p x) -> p x", p=128),
            )
            if mode == "load":
                return
            o = opool.tile([128, FD], FP32, tag="o")
            if mode == "dma":
                eng_out.dma_start(
                    out=of[b, :].rearrange("(p x) -> p x", p=128), in_=qt[:, :]
                )
                return

            # octet-boundary shift: aux[p] = qt[p-1, ts=7 chunk] (row 0 = 0)
            aux = auxp.tile([128, JC], FP32, tag="aux")
            nc.tensor.matmul(
                aux[:, :],
                lhsT=smatSB[:, :],
                rhs=qt[:, FD - JC:FD],
                start=True,
                stop=True,
            )

            # prod: o = q * q_shifted  (DVE)
            if probe != "noprod":
                nc.vector.tensor_tensor(
                    out=o[:, JC:FD], in0=qt[:, JC:FD], in1=qt[:, 0:FD - JC],
                    op=Alu.mult,
                )
                nc.vector.tensor_tensor(
                    out=o[:, 0:JC], in0=qt[:, 0:JC], in1=aux[:, :],
                    op=Alu.mult,
                )
                osrc = o
            else:
                osrc = qt

            if dred:
                # d = reduce over c in one DVE pass, written in (j, ts) order
                d = spool.tile([128, SD], FP32, tag="d")
                nc.vector.tensor_reduce(
                    out=_ap(d, [[1, TS], [TS, J]]),
                    in_=osrc.rearrange("p (ts j c) -> p ts j c", j=J, c=C),
                    axis=mybir.AxisListType.X, op=Alu.add,
                )
                sg = spool.tile([128, SD], FP32, tag="sg")
                nc.scalar.activation(sg[:, :], d[:, :], Act.Sign, scale=-1.0)
                e = spool.tile([128, SD], BF16, tag="e")
                nc.scalar.activation(e[:, :], sg[:, :], Act.Relu)
            if not dred:
                # dot over c, pairwise (c0+c1)+(c2+c3); d in (j, ts) order
                u = spool.tile([128, 2 * SD], FP32, tag="u")
                ov = osrc.rearrange("p (s c) -> p s c", c=C)
                uv = u.rearrange("p (s k) -> p s k", k=2)
                opairs = ov.rearrange("p s (k two) -> p s k two", k=2)
                if probe == "noud":
                    pass
                elif u_pool:
                    nc.gpsimd.tensor_tensor(
                        out=uv[:, 0:u_pool], in0=opairs[:, 0:u_pool, :, 0],
                        in1=opairs[:, 0:u_pool, :, 1], op=Alu.add,
                    )
                    eng_u.tensor_tensor(
                        out=uv[:, u_pool:SD], in0=opairs[:, u_pool:SD, :, 0],
                        in1=opairs[:, u_pool:SD, :, 1], op=Alu.add,
                    )
                elif u2:
                    # u in (k, s) split-half layout: strided write, but d
                    # then reads two contiguous 512-elem halves.
                    nc.vector.tensor_tensor(
                        out=bass.AP(
                            tensor=u.tensor, offset=u.offset,
                            ap=[list(u.ap[0]), [1, SD], [SD, 2]],
                        ),
                        in0=opairs[:, :, :, 0], in1=opairs[:, :, :, 1],
                        op=Alu.add,
                    )
                else:
                    eng_u.tensor_tensor(
                        out=uv, in0=opairs[:, :, :, 0], in1=opairs[:, :, :, 1],
                        op=Alu.add,
                    )
                if u2:
                    u_k = u.rearrange("p (k ts j) -> p ts j k", j=J, k=2)
                else:
                    u_k = u.rearrange("p (ts j k) -> p ts j k", j=J, k=2)
            if dred:
                pass
            elif probe == "noud":
                d = spool.tile([128, SD], FP32, tag="d")
                sg = spool.tile([128, SD], FP32, tag="sg")
                nc.scalar.activation(sg[:, :], amask[:, 0:SD], Act.Sign,
                                     scale=-1.0)
                e = spool.tile([128, SD], BF16, tag="e")
                nc.scalar.activation(e[:, :], sg[:, :], Act.Relu)
            elif fuse_e:
                # e = 1.0 iff (-u0 > u1) <=> d = u0 + u1 < 0, one DVE/Pool
                # op replacing the d add + ACT Sign + ACT Relu chain.  The
                # comparison of the two fp values is exact, and for near-tie
                # sums fl(u0+u1) is exact (Sterbenz), so this matches the
                # old fl(d) < 0 test bit-for-bit.
                e = spool.tile([128, SD], BF16, tag="e")
                eng_e.scalar_tensor_tensor(
                    out=_ap(e, [[1, TS], [TS, J]]),
                    in0=u_k[:, :, :, 0], scalar=-1.0, in1=u_k[:, :, :, 1],
                    op0=Alu.mult, op1=Alu.is_gt,
                )
            else:
                d = spool.tile([128, SD], FP32, tag="d")  # (j, ts) layout
                eng_d.tensor_tensor(
                    out=_ap(d, [[1, TS], [TS, J]]),
                    in0=u_k[:, :, :, 0],
                    in1=u_k[:, :, :, 1],
                    op=Alu.add,
                )

                # flip indicator e = Relu(Sign(-d)), bf16, (j, ts) layout
                sg = spool.tile([128, SD], FP32, tag="sg")
                nc.scalar.activation(sg[:, :], d[:, :], Act.Sign, scale=-1.0)
                e = spool.tile([128, SD], BF16, tag="e")
                nc.scalar.activation(e[:, :], sg[:, :], Act.Relu)
            # t=0 has no flip (also guards Sign(0) semantics)
            nc.scalar.mul(
                e.rearrange("p (j ts) -> p j ts", ts=TS)[0:1, :, 0],
                e.rearrange("p (j ts) -> p j ts", ts=TS)[0:1, :, 0],
                0.0,
            )

            # within-octet inclusive prefix PARITY (segmented xor-scan):
            # state = (mask * state) xor e  -> 0/1 running parity per joint
            rowp = spool.tile([128, SD], FP32, tag="rowp")
            if probe == "noscan":
                nc.vector.tensor_copy(out=rowp[:, :], in_=e[:, :])
            else:
                eng_scan.tensor_tensor_scan(
                    out=rowp[:, :], data0=amask[:, :], data1=e[:, :],
                    initial=0.0, op0=Alu.mult, op1=Alu.logical_xor,
                )

            # octet-level: count of odd rows above (parity-sum via matmul)
            offs = offp.tile([128, J], FP32, tag="offs")
            nc.tensor.matmul(
                offs[:, :],
                lhsT=pmatSB[:, :],
                rhs=rowp.rearrange("p (j ts) -> p j ts", ts=TS)[:, :, 7],
                start=True,
                stop=True,
            )
            if sigm_xor:
                # pre = rowp XOR oddbit (0/1 int32, (j, ts) layout); then
                # sigm = pre * 0x80008000 = sign-bit mask for a packed
                # bf16 pair.  Final mult becomes an int32 XOR at HALF the
                # element count (two bf16 share one sigma).
                offi = spool.tile([128, J], I32, tag="offi")
                nc.vector.tensor_copy(out=offi[:, :], in_=offs[:, :])
                offb = spool.tile([128, J], I32, tag="offb")
                nc.vector.tensor_scalar(
                    out=offb[:, :], in0=offi[:, :], scalar1=1, scalar2=None,
                    op0=Alu.bitwise_and,
                )
                pre = spool.tile([128, SD], I32, tag="pre")
                nc.vector.tensor_tensor(
                    out=pre.rearrange("p (j ts) -> p j ts", ts=TS),
                    in0=rowp.rearrange("p (j ts) -> p j ts", ts=TS),
                    in1=_ap(offb, [[1, J], [0, TS]]),
                    op=Alu.logical_xor,
                )
                sigm = spool.tile([128, SD], I32, tag="sigm")
                nc.vector.tensor_tensor(
                    out=sigm[:, :], in0=pre[:, :],
                    in1=_ap(msk32, [[0, SD]]), op=Alu.mult,
                )
                qb = bopool.tile([128, FD], BF16, tag="qb")
                nc.scalar.copy(qb[:, :], qt[:, :])
                dst = bopool.tile([128, FD], BF16, tag="ob")
                qb3 = qb[:, :].bitcast(I32).rearrange(
                    "p (ts j cc) -> p ts j cc", ts=TS, j=J)
                ob3 = dst[:, :].bitcast(I32).rearrange(
                    "p (ts j cc) -> p ts j cc", ts=TS, j=J)
                nc.vector.tensor_tensor(
                    out=ob3, in0=qb3,
                    in1=bass.AP(
                        tensor=sigm.tensor, offset=sigm.offset,
                        ap=[list(sigm.ap[0]), [1, TS], [TS, J], [0, 2]],
                    ),
                    op=Alu.bitwise_xor,
                )
                eng_out.dma_start(
                    out=of[b, :].rearrange("(p x) -> p x", p=128),
                    in_=dst[:, :],
                )
                return

            # parity of that count -> sigma_off in {+1, -1} per (p, j)
            sigo = spool.tile([128, J], BF16, tag="sigo")
            if sigo_cos:
                # count mod 2 on DVE (exact: counts are integer-valued fp32),
                # then the +-1 affine on ACT.
                offm = spool.tile([128, J], FP32, tag="offm")
                nc.vector.tensor_scalar(
                    out=offm[:, :], in0=offs[:, :], scalar1=2.0, scalar2=None,
                    op0=Alu.mod,
                )
                nc.scalar.activation(sigo[:, :], offm[:, :], Act.Copy,
                                     bias=1.0, scale=-2.0)
            else:
                offi = spool.tile([128, J], I32, tag="offi")
                nc.vector.tensor_copy(out=offi[:, :], in_=offs[:, :])
                offb = spool.tile([128, J], I32, tag="offb")
                nc.vector.tensor_scalar(
                    out=offb[:, :], in0=offi[:, :], scalar1=1, scalar2=None,
                    op0=Alu.bitwise_and,
                )
                nc.scalar.activation(sigo[:, :], offb[:, :], Act.Copy,
                                     bias=1.0, scale=-2.0)
            # sigma_row in {+1, -1} from the 0/1 row parity  (ACT)
            sigr = spool.tile([128, SD], BF16, tag="sigr")
            nc.scalar.activation(sigr[:, :], rowp[:, :], Act.Copy,
                                 bias=1.0, scale=-2.0)
            # sigma = sigma_row * sigma_off, (j, ts) layout
            sig = spool.tile([128, SD], BF16, tag="sig")
            eng_sig.tensor_tensor(
                out=sig.rearrange("p (j ts) -> p j ts", ts=TS),
                in0=sigr.rearrange("p (j ts) -> p j ts", ts=TS),
                in1=_ap(sigo, [[1, J], [0, TS]]),
                op=Alu.mult,
            )

            # out = q * sigma (broadcast over c), exact +/-1 multiply;
            # split by ts-range between VectorE and GpSimd.  With out_bf16
            # the result is written (rounded) to a bf16 tile and the store
            # moves half the bytes.
            if out_bf16:
                dst = bopool.tile([128, FD], BF16, tag="ob")
            else:
                dst = o
            if out_bf16 and cast_act:
                # bf16 copy of q on ACT so the final mult runs 16-bit on DVE
                qb = bopool.tile([128, FD], BF16, tag="qb")
                nc.scalar.copy(qb[:, :], qt[:, :])
                qsrc = qb
            else:
                qsrc = qt
            qv = qsrc.rearrange("p (ts x) -> p ts x", ts=TS)
            ow = dst.rearrange("p (ts x) -> p ts x", ts=TS)
            tsplit = mult_split
            if probe == "halfmult":
                tsplit = 4
                nc.vector.tensor_tensor(
                    out=ow[:, 0:4, :],
                    in0=qv[:, 0:4, :],
                    in1=bass.AP(
                        tensor=sig.tensor, offset=sig.offset,
                        ap=[list(sig.ap[0]), [1, 4], [TS, J], [0, C]],
                    ),
                    op=Alu.mult,
                )
            elif tsplit > 0:
                nc.vector.tensor_tensor(
                    out=ow[:, 0:tsplit, :],
                    in0=qv[:, 0:tsplit, :],
                    in1=bass.AP(
                        tensor=sig.tensor, offset=sig.offset,
                        ap=[list(sig.ap[0]), [1, tsplit], [TS, J], [0, C]],
                    ),
                    op=Alu.mult,
                )
            if tsplit < TS:
                nc.gpsimd.tensor_tensor(
                    out=ow[:, tsplit:TS, :],
                    in0=qv[:, tsplit:TS, :],
                    in1=bass.AP(
                        tensor=sig.tensor, offset=sig.offset + tsplit,
                        ap=[list(sig.ap[0]), [1, TS - tsplit], [TS, J],
                            [0, C]],
                    ),
                    op=Alu.mult,
                )

            eng_out.dma_start(
                out=of[b, :].rearrange("(p x) -> p x", p=128), in_=dst[:, :]
            )

        if unroll:
            for _ in range(reps):
                emit_body()
        elif reps == 1:
            emit_body()
        else:
            assert reps % loop_unroll == 0
            with tc.For_i(0, reps // loop_unroll, 1,
                          staggered_reset=bool(staggered)):
                for _ in range(loop_unroll):
                    emit_body()
    return nc


def make_consts():
    smat = np.eye(128, k=1, dtype=np.float32)       # S[k, m] = 1 iff m == k+1
    pmat = np.triu(np.ones((128, 128), np.float32), k=1)  # strict prefix
    return smat, pmat


def make_in_maps(q, smat, pmat):
    return [
        {"q": q[c * BPC:(c + 1) * BPC], "smat": smat, "pmat": pmat}
        for c in range(NCORES)
    ]


def kernel(joint_rotations: np.ndarray) -> np.ndarray:
    q = np.ascontiguousarray(joint_rotations, dtype=np.float32)
    assert q.shape == (B, T, J, C)
    smat, pmat = make_consts()
    nc = build_nc()
    nc.finalize()   # run bacc passes (wait splitting, reg alloc) + freeze
    in_maps = make_in_maps(q, smat, pmat)
    res = run_bass_kernel_spmd(nc, in_maps, list(range(NCORES)))
    outs = [np.asarray(r["out"]).astype(np.float32) for r in res.results]
    return np.concatenate(outs, axis=0)



# revision 25
# speedup vs baseline: 3.0257x; 1.0203x over previous
"""Trainium2 Bass kernel for BatchRemoveQuatDiscontinuities.

Algorithm (per (batch, joint) lane):
    d[t]    = dot(q[t], q[t-1])                (fp32, 4-wide dot)
    flip[t] = 1 if d[t] < 0 else 0             (t >= 1; flip[0] = 0)
    sigma[t] = (-1)^(sum_{s<=t} flip[s])       (cumulative sign parity)
    out[t]  = q[t] * sigma[t]

Mapping on a NeuronCore (data-parallel over batch across 8 cores):
  * One tile = one batch clip, loaded as a single fully-contiguous 1MB
    DMA: [128 partitions = t/8, free = (ts: 8, j: 64, c: 4)].  Loads on
    the SP HWDGE ring (nc.sync), stores on the ACT ring (nc.scalar).
  * q[t-1]: within a partition it is a free-axis offset (-256); the
    octet boundary (ts=0) needs q[p-1, ts=7], produced by a TensorE
    matmul with an off-diagonal 0/1 matrix S into PSUM (fp32 exact).
  * prod on DVE, 4-wide dot via two pairwise adds (c0+c1)+(c2+c3),
    d written in (j, ts) order; flip indicator e = Relu(Sign(-d)) on
    ScalarE (bf16).
  * Within-octet inclusive prefix: tensor_tensor_scan with a reset mask
    (state = mask*state xor e), segments of 8 per joint.  Octet-level
    parity via strict-triangular matmul over partitions on the per-row
    totals; parity of the count -> sigo (int &1, ACT affine to +-1).
  * sigma_row = 1-2*rowp on ScalarE (bf16); sig = sigr*sigo on GpSimd.
    Final out = q * sig (broadcast over c), ts-split DVE/GpSimd by
    mult_split - exact +/-1 multiply.

Engine budget (HW-measured, ablation-profiled): DVE is 100% critical at
~1.1-1.45ns/free-elem fp32 (prod 36us + u/d 36us + scan 11us + final
mult 36us per rep of 16 clips); 16-bit gives NO 2x here (broadcast
in1 AP blocks perf mode).  GpSimd TT hurts at ANY dose (ms7 = +4us) -
keep Pool idle.  ScalarE has ~60us slack and absorbs the bf16 cast.
DMA: load-only 45.6us/16MB (351GB/s), aggregate ~330GB/s/core; at 24MB
(fp32 in + bf16 out) DMA is NOT binding.  tensor_reduce(X) runs at
input rate (no win over pairwise adds); scalar_tensor_tensor is_gt is
~4x slower than plain TT (don't fuse e); tensor_tensor_scan does not
lower on Pool.  tc.For_i costs ~14us/iteration (all-engine barrier at
the back edge; staggered_reset no help) - amortize with loop_unroll.
Best config: out_bf16=1 cast_act=1 mult_split=8 sigm_xor=1 (body
~130us vs 165us for the old ms4 config, which was Pool-bound at 16K
elems ~ 10ns/elem).  sigm_xor: two bf16 outputs share one sigma, so the
final multiply is an int32 XOR of packed sign bits (0x80008000 mask) at
HALF the element count - TT fp32/broadcast never gets DVE 2x (only
2x_1p exists for TT and it needs all-packed 2-byte operands), so
shrinking element count is the only lever.
"""

import numpy as np
from contextlib import ExitStack

import concourse.bass as bass
import concourse.bacc as bacc
import concourse.tile as tile
from concourse import mybir
from concourse.bass_utils import run_bass_kernel_spmd

B, T, J, C = 128, 1024, 64, 4
NCORES = 8
JC = J * C                      # 256 floats per t
BPC = B // NCORES               # 16 batch clips per core
TS = 8                          # t per partition (octet)
FD = TS * JC                    # tile free dim = 2048 floats
SD = J * TS                     # prefix free dim = 512 (j, ts)

FP32 = mybir.dt.float32
BF16 = mybir.dt.bfloat16
I32 = mybir.dt.int32
Alu = mybir.AluOpType
Act = mybir.ActivationFunctionType


def _ap(apx, dims):
    """AP with explicit [step, count] free dims appended to partition dim."""
    return bass.AP(
        tensor=apx.tensor, offset=apx.offset,
        ap=[list(apx.ap[0]), *[list(d) for d in dims]],
    )


def build_nc(bpc=BPC, t=T, reps=1, mode="full", mult_split=8,
             sig_eng="dve", d_eng="dve", u_eng="dve", scan_eng="dve",
             out_ring="sync", qbufs=8, obufs=4, sbufs=4, sigo_cos=0,
             out_bf16=1, bobufs=5, cp=1, unroll=0, loop_unroll=1,
             staggered=0, fuse_e=0, e_eng="dve", cast_act=1, u_pool=0,
             probe="none", dred=0, sigm_xor=1, u2=0):
    assert t % (128 * TS) == 0
    nc = bacc.Bacc(None, target_bir_lowering=False)
    q = nc.declare_dram_parameter("q", [bpc, t, J, C], FP32, isOutput=False)
    smat = nc.declare_dram_parameter("smat", [128, 128], FP32, isOutput=False)
    pmat = nc.declare_dram_parameter("pmat", [128, 128], FP32, isOutput=False)
    out_dt = BF16 if out_bf16 else FP32
    out = nc.declare_dram_parameter("out", [bpc, t, J, C], out_dt,
                                    isOutput=True)
    qf = q.rearrange("b t j c -> b (t j c)")
    of = out.rearrange("b t j c -> b (t j c)")

    eng_sig = nc.gpsimd if sig_eng == "pool" else nc.vector
    eng_d = nc.gpsimd if d_eng == "pool" else nc.vector
    eng_u = nc.gpsimd if u_eng == "pool" else nc.vector
    eng_scan = nc.gpsimd if scan_eng == "pool" else nc.vector
    eng_e = nc.gpsimd if e_eng == "pool" else nc.vector
    eng_out = nc.scalar if out_ring == "act" else nc.sync

    with tile.TileContext(nc) as tc, ExitStack() as ctx:
        consts = ctx.enter_context(tc.tile_pool(name="consts", bufs=1))
        qpool = ctx.enter_context(tc.tile_pool(name="qpool", bufs=qbufs))
        opool = ctx.enter_context(tc.tile_pool(name="opool", bufs=obufs))
        bopool = (ctx.enter_context(tc.tile_pool(name="bopool", bufs=bobufs))
                  if out_bf16 else None)
        spool = ctx.enter_context(tc.tile_pool(name="spool", bufs=sbufs))
        auxp = ctx.enter_context(tc.tile_pool(name="auxp", bufs=4, space="PSUM"))
        offp = ctx.enter_context(tc.tile_pool(name="offp", bufs=4, space="PSUM"))

        smatSB = consts.tile([128, 128], FP32)
        nc.sync.dma_start(out=smatSB[:, :], in_=smat[:, :])
        pmatSB = consts.tile([128, 128], FP32)
        nc.sync.dma_start(out=pmatSB[:, :], in_=pmat[:, :])
        amask = consts.tile([128, cp * SD], FP32)
        nc.vector.memset(amask[:, :], 1.0)
        nc.vector.memset(
            amask.rearrange("p (b j ts) -> p b j ts", b=cp, ts=TS)[:, :, :, 0],
            0.0,
        )
        pihalf = consts.tile([128, 1], FP32)
        nc.vector.memset(pihalf[:, :], float(np.pi / 2))
        msk32 = consts.tile([128, 1], I32)
        # sign bits of a packed bf16 pair: 0x80008000 as int32
        nc.vector.memset(msk32[:, :], -2147450880)
        mskf = consts.tile([128, SD], I32)
        nc.vector.memset(mskf[:, :], -2147450880)

        def emit_body():
            if cp == 1:
                for b in range(bpc):
                    emit_tile(b)
            else:
                for g in range(bpc // cp):
                    emit_tile_cp2(g)

        def emit_tile_cp2(g):
            """Two clips per tile: same per-engine work doses as the cp=1
            path (Pool only gets the final-mult halves), but the 1024-wide
            stages (prod/scan/sig/ACT chain) are fused across the clip pair
            to halve instruction counts, and the DMAs are 2MB."""
            qt = qpool.tile([128, cp, FD], FP32, tag="qt")
            nc.sync.dma_start(
                out=qt[:, :, :],
                in_=qf[g * cp:(g + 1) * cp, :].rearrange(
                    "b (p x) -> p b x", p=128),
            )
            o = opool.tile([128, cp, FD], FP32, tag="o")
            aux = auxp.tile([128, cp, JC], FP32, tag="aux")
            nc.tensor.matmul(
                aux[:, :, :], lhsT=smatSB[:, :], rhs=qt[:, :, FD - JC:FD],
                start=True, stop=True,
            )
            nc.vector.tensor_tensor(
                out=o[:, :, JC:FD], in0=qt[:, :, JC:FD],
                in1=qt[:, :, 0:FD - JC], op=Alu.mult,
            )
            nc.vector.tensor_tensor(
                out=o[:, :, 0:JC], in0=qt[:, :, 0:JC], in1=aux[:, :, :],
                op=Alu.mult,
            )
            u = spool.tile([128, cp, 2 * SD], FP32, tag="u")
            d = spool.tile([128, cp, SD], FP32, tag="d")
            for c2 in range(cp):
                opairs = o.rearrange(
                    "p b (s k two) -> p b s k two", k=2, two=2)[:, c2]
                uv = u.rearrange("p b (s k) -> p b s k", k=2)[:, c2]
                nc.vector.tensor_tensor(
                    out=uv, in0=opairs[:, :, :, 0], in1=opairs[:, :, :, 1],
                    op=Alu.add,
                )
                u_k = u.rearrange(
                    "p b (ts j k) -> p b ts j k", j=J, k=2)[:, c2]
                dv = bass.AP(
                    tensor=d.tensor, offset=d.offset + c2 * SD,
                    ap=[list(d.ap[0]), [1, TS], [TS, J]],
                )
                nc.vector.tensor_tensor(
                    out=dv, in0=u_k[:, :, :, 0], in1=u_k[:, :, :, 1],
                    op=Alu.add,
                )
            df = d.rearrange("p b s -> p (b s)")
            sg = spool.tile([128, cp * SD], FP32, tag="sg")
            nc.scalar.activation(sg[:, :], df, Act.Sign, scale=-1.0)
            e = spool.tile([128, cp * SD], BF16, tag="e")
            nc.scalar.activation(e[:, :], sg[:, :], Act.Relu)
            ev = e.rearrange("p (b j ts) -> p b j ts", b=cp, ts=TS)
            nc.scalar.mul(ev[0:1, :, :, 0], ev[0:1, :, :, 0], 0.0)
            rowp = spool.tile([128, cp * SD], FP32, tag="rowp")
            nc.vector.tensor_tensor_scan(
                out=rowp[:, :], data0=amask[:, :], data1=e[:, :],
                initial=0.0, op0=Alu.mult, op1=Alu.logical_xor,
            )
            offs = offp.tile([128, cp, J], FP32, tag="offs")
            rr = rowp.rearrange("p (b j ts) -> p b j ts", b=cp, ts=TS)
            nc.tensor.matmul(
                offs[:, :, :], lhsT=pmatSB[:, :], rhs=rr[:, :, :, 7],
                start=True, stop=True,
            )
            offi = spool.tile([128, cp * J], I32, tag="offi")
            nc.vector.tensor_copy(
                out=offi[:, :], in_=offs.rearrange("p b j -> p (b j)"))
            offb = spool.tile([128, cp * J], I32, tag="offb")
            nc.vector.tensor_scalar(
                out=offb[:, :], in0=offi[:, :], scalar1=1, scalar2=None,
                op0=Alu.bitwise_and,
            )
            sigo = spool.tile([128, cp * J], BF16, tag="sigo")
            nc.scalar.activation(sigo[:, :], offb[:, :], Act.Copy,
                                 bias=1.0, scale=-2.0)
            sigr = spool.tile([128, cp * SD], BF16, tag="sigr")
            nc.scalar.activation(sigr[:, :], rowp[:, :], Act.Copy,
                                 bias=1.0, scale=-2.0)
            sig = spool.tile([128, cp * SD], BF16, tag="sig")
            nc.vector.tensor_tensor(
                out=sig.rearrange("p (bj ts) -> p bj ts", ts=TS),
                in0=sigr.rearrange("p (bj ts) -> p bj ts", ts=TS),
                in1=_ap(sigo, [[1, cp * J], [0, TS]]),
                op=Alu.mult,
            )
            if out_bf16:
                dst = bopool.tile([128, cp, FD], BF16, tag="ob")
            else:
                dst = o
            if out_bf16 and cast_act:
                qb = bopool.tile([128, cp, FD], BF16, tag="qb")
                nc.scalar.copy(qb[:, :, :], qt[:, :, :])
                qsrc = qb
            else:
                qsrc = qt
            for c2 in range(cp):
                qv = qsrc.rearrange("p b (ts x) -> p b ts x", ts=TS)[:, c2]
                ow = dst.rearrange("p b (ts x) -> p b ts x", ts=TS)[:, c2]
                sbase = sig.offset + c2 * SD
                if mult_split > 0:
                    nc.vector.tensor_tensor(
                        out=ow[:, 0:mult_split, :],
                        in0=qv[:, 0:mult_split, :],
                        in1=bass.AP(
                            tensor=sig.tensor, offset=sbase,
                            ap=[list(sig.ap[0]), [1, mult_split], [TS, J],
                                [0, C]],
                        ),
                        op=Alu.mult,
                    )
                if mult_split < TS:
                    nc.gpsimd.tensor_tensor(
                        out=ow[:, mult_split:TS, :],
                        in0=qv[:, mult_split:TS, :],
                        in1=bass.AP(
                            tensor=sig.tensor, offset=sbase + mult_split,
                            ap=[list(sig.ap[0]), [1, TS - mult_split],
                                [TS, J], [0, C]],
                        ),
                        op=Alu.mult,
                    )
            eng_out.dma_start(
                out=of[g * cp:(g + 1) * cp, :].rearrange(
                    "b (p x) -> p b x", p=128),
                in_=dst[:, :, :],
            )

        stile = None
        if mode == "store":
            stile = consts.tile([128, FD], FP32)
            nc.vector.memset(stile[:, :], 1.0)

        def emit_tile(b):
            if mode == "store":
                eng_out.dma_start(
                    out=of[b, :].rearrange("(p x) -> p x", p=128),
                    in_=stile[:, :],
                )
                return
            qt = qpool.tile([128, FD], FP32, tag="qt")
            nc.sync.dma_start(
                out=qt[:, :],
                in_=qf[b, :].rearrange("(p x) -> p x", p=128),
            )
            if mode == "load":
                return
            o = opool.tile([128, FD], FP32, tag="o")
            if mode == "dma":
                eng_out.dma_start(
                    out=of[b, :].rearrange("(p x) -> p x", p=128), in_=qt[:, :]
                )
                return

            # octet-boundary shift: aux[p] = qt[p-1, ts=7 chunk] (row 0 = 0)
            aux = auxp.tile([128, JC], FP32, tag="aux")
            nc.tensor.matmul(
                aux[:, :],
                lhsT=smatSB[:, :],
                rhs=qt[:, FD - JC:FD],
                start=True,
                stop=True,
            )

            # prod: o = q * q_shifted  (DVE)
            if probe != "noprod":
                nc.vector.tensor_tensor(
                    out=o[:, JC:FD], in0=qt[:, JC:FD], in1=qt[:, 0:FD - JC],
                    op=Alu.mult,
                )
                nc.vector.tensor_tensor(
                    out=o[:, 0:JC], in0=qt[:, 0:JC], in1=aux[:, :],
                    op=Alu.mult,
                )
                osrc = o
            else:
                osrc = qt

            if dred:
                # d = reduce over c in one DVE pass, written in (j, ts) order
                d = spool.tile([128, SD], FP32, tag="d")
                nc.vector.tensor_reduce(
                    out=_ap(d, [[1, TS], [TS, J]]),
                    in_=osrc.rearrange("p (ts j c) -> p ts j c", j=J, c=C),
                    axis=mybir.AxisListType.X, op=Alu.add,
                )
                sg = spool.tile([128, SD], FP32, tag="sg")
                nc.scalar.activation(sg[:, :], d[:, :], Act.Sign, scale=-1.0)
                e = spool.tile([128, SD], BF16, tag="e")
                nc.scalar.activation(e[:, :], sg[:, :], Act.Relu)
            if not dred:
                # dot over c, pairwise (c0+c1)+(c2+c3); d in (j, ts) order
                u = spool.tile([128, 2 * SD], FP32, tag="u")
                ov = osrc.rearrange("p (s c) -> p s c", c=C)
                uv = u.rearrange("p (s k) -> p s k", k=2)
                opairs = ov.rearrange("p s (k two) -> p s k two", k=2)
                if probe == "noud":
                    pass
                elif u_pool:
                    nc.gpsimd.tensor_tensor(
                        out=uv[:, 0:u_pool], in0=opairs[:, 0:u_pool, :, 0],
                        in1=opairs[:, 0:u_pool, :, 1], op=Alu.add,
                    )
                    eng_u.tensor_tensor(
                        out=uv[:, u_pool:SD], in0=opairs[:, u_pool:SD, :, 0],
                        in1=opairs[:, u_pool:SD, :, 1], op=Alu.add,
                    )
                elif u2:
                    # u in (k, s) split-half layout: strided write, but d
                    # then reads two contiguous 512-elem halves.
                    nc.vector.tensor_tensor(
                        out=bass.AP(
                            tensor=u.tensor, offset=u.offset,
                            ap=[list(u.ap[0]), [1, SD], [SD, 2]],
                        ),
                        in0=opairs[:, :, :, 0], in1=opairs[:, :, :, 1],
                        op=Alu.add,
                    )
                else:
                    eng_u.tensor_tensor(
                        out=uv, in0=opairs[:, :, :, 0], in1=opairs[:, :, :, 1],
                        op=Alu.add,
                    )
                if u2:
                    u_k = u.rearrange("p (k ts j) -> p ts j k", j=J, k=2)
                else:
                    u_k = u.rearrange("p (ts j k) -> p ts j k", j=J, k=2)
            if dred:
                pass
            elif probe == "noud":
                d = spool.tile([128, SD], FP32, tag="d")
                sg = spool.tile([128, SD], FP32, tag="sg")
                nc.scalar.activation(sg[:, :], amask[:, 0:SD], Act.Sign,
                                     scale=-1.0)
                e = spool.tile([128, SD], BF16, tag="e")
                nc.scalar.activation(e[:, :], sg[:, :], Act.Relu)
            elif fuse_e:
                # e = 1.0 iff (-u0 > u1) <=> d = u0 + u1 < 0, one DVE/Pool
                # op replacing the d add + ACT Sign + ACT Relu chain.  The
                # comparison of the two fp values is exact, and for near-tie
                # sums fl(u0+u1) is exact (Sterbenz), so this matches the
                # old fl(d) < 0 test bit-for-bit.
                e = spool.tile([128, SD], BF16, tag="e")
                eng_e.scalar_tensor_tensor(
                    out=_ap(e, [[1, TS], [TS, J]]),
                    in0=u_k[:, :, :, 0], scalar=-1.0, in1=u_k[:, :, :, 1],
                    op0=Alu.mult, op1=Alu.is_gt,
                )
            else:
                d = spool.tile([128, SD], FP32, tag="d")  # (j, ts) layout
                eng_d.tensor_tensor(
                    out=_ap(d, [[1, TS], [TS, J]]),
                    in0=u_k[:, :, :, 0],
                    in1=u_k[:, :, :, 1],
                    op=Alu.add,
                )

                # flip indicator e = Relu(Sign(-d)), bf16, (j, ts) layout
                sg = spool.tile([128, SD], FP32, tag="sg")
                nc.scalar.activation(sg[:, :], d[:, :], Act.Sign, scale=-1.0)
                e = spool.tile([128, SD], BF16, tag="e")
                nc.scalar.activation(e[:, :], sg[:, :], Act.Relu)
            # t=0 has no flip (also guards Sign(0) semantics)
            nc.scalar.mul(
                e.rearrange("p (j ts) -> p j ts", ts=TS)[0:1, :, 0],
                e.rearrange("p (j ts) -> p j ts", ts=TS)[0:1, :, 0],
                0.0,
            )

            # within-octet inclusive prefix PARITY (segmented xor-scan):
            # state = (mask * state) xor e  -> 0/1 running parity per joint
            rowp = spool.tile([128, SD], FP32, tag="rowp")
            if probe == "noscan":
                nc.vector.tensor_copy(out=rowp[:, :], in_=e[:, :])
            else:
                eng_scan.tensor_tensor_scan(
                    out=rowp[:, :], data0=amask[:, :], data1=e[:, :],
                    initial=0.0, op0=Alu.mult, op1=Alu.logical_xor,
                )

            # octet-level: count of odd rows above (parity-sum via matmul)
            offs = offp.tile([128, J], FP32, tag="offs")
            nc.tensor.matmul(
                offs[:, :],
                lhsT=pmatSB[:, :],
                rhs=rowp.rearrange("p (j ts) -> p j ts", ts=TS)[:, :, 7],
                start=True,
                stop=True,
            )
            if sigm_xor:
                # pre = rowp XOR oddbit (0/1 int32, (j, ts) layout); then
                # sigm = pre * 0x80008000 = sign-bit mask for a packed
                # bf16 pair.  Final mult becomes an int32 XOR at HALF the
                # element count (two bf16 share one sigma).
                offi = spool.tile([128, J], I32, tag="offi")
                nc.vector.tensor_copy(out=offi[:, :], in_=offs[:, :])
                offb = spool.tile([128, J], I32, tag="offb")
                nc.vector.tensor_scalar(
                    out=offb[:, :], in0=offi[:, :], scalar1=1, scalar2=None,
                    op0=Alu.bitwise_and,
                )
                pre = spool.tile([128, SD], I32, tag="pre")
                nc.vector.tensor_tensor(
                    out=pre.rearrange("p (j ts) -> p j ts", ts=TS),
                    in0=rowp.rearrange("p (j ts) -> p j ts", ts=TS),
                    in1=_ap(offb, [[1, J], [0, TS]]),
                    op=Alu.logical_xor,
                )
                sigm = spool.tile([128, SD], I32, tag="sigm")
                # full-width const mask: packed in1 (stride-0 broadcast APs
                # run in the slower DVE class)
                nc.vector.tensor_tensor(
                    out=sigm[:, :], in0=pre[:, :],
                    in1=mskf[:, :], op=Alu.mult,
                )
                qb = bopool.tile([128, FD], BF16, tag="qb")
                nc.scalar.copy(qb[:, :], qt[:, :])
                dst = bopool.tile([128, FD], BF16, tag="ob")
                qb3 = qb[:, :].bitcast(I32).rearrange(
                    "p (ts j cc) -> p ts j cc", ts=TS, j=J)
                ob3 = dst[:, :].bitcast(I32).rearrange(
                    "p (ts j cc) -> p ts j cc", ts=TS, j=J)
                nc.vector.tensor_tensor(
                    out=ob3, in0=qb3,
                    in1=bass.AP(
                        tensor=sigm.tensor, offset=sigm.offset,
                        ap=[list(sigm.ap[0]), [1, TS], [TS, J], [0, 2]],
                    ),
                    op=Alu.bitwise_xor,
                )
                eng_out.dma_start(
                    out=of[b, :].rearrange("(p x) -> p x", p=128),
                    in_=dst[:, :],
                )
                return

            # parity of that count -> sigma_off in {+1, -1} per (p, j)
            sigo = spool.tile([128, J], BF16, tag="sigo")
            if sigo_cos:
                # count mod 2 on DVE (exact: counts are integer-valued fp32),
                # then the +-1 affine on ACT.
                offm = spool.tile([128, J], FP32, tag="offm")
                nc.vector.tensor_scalar(
                    out=offm[:, :], in0=offs[:, :], scalar1=2.0, scalar2=None,
                    op0=Alu.mod,
                )
                nc.scalar.activation(sigo[:, :], offm[:, :], Act.Copy,
                                     bias=1.0, scale=-2.0)
            else:
                offi = spool.tile([128, J], I32, tag="offi")
                nc.vector.tensor_copy(out=offi[:, :], in_=offs[:, :])
                offb = spool.tile([128, J], I32, tag="offb")
                nc.vector.tensor_scalar(
                    out=offb[:, :], in0=offi[:, :], scalar1=1, scalar2=None,
                    op0=Alu.bitwise_and,
                )
                nc.scalar.activation(sigo[:, :], offb[:, :], Act.Copy,
                                     bias=1.0, scale=-2.0)
            # sigma_row in {+1, -1} from the 0/1 row parity  (ACT)
            sigr = spool.tile([128, SD], BF16, tag="sigr")
            nc.scalar.activation(sigr[:, :], rowp[:, :], Act.Copy,
                                 bias=1.0, scale=-2.0)
            # sigma = sigma_row * sigma_off, (j, ts) layout
            sig = spool.tile([128, SD], BF16, tag="sig")
            eng_sig.tensor_tensor(
                out=sig.rearrange("p (j ts) -> p j ts", ts=TS),
                in0=sigr.rearrange("p (j ts) -> p j ts", ts=TS),
                in1=_ap(sigo, [[1, J], [0, TS]]),
                op=Alu.mult,
            )

            # out = q * sigma (broadcast over c), exact +/-1 multiply;
            # split by ts-range between VectorE and GpSimd.  With out_bf16
            # the result is written (rounded) to a bf16 tile and the store
            # moves half the bytes.
            if out_bf16:
                dst = bopool.tile([128, FD], BF16, tag="ob")
            else:
                dst = o
            if out_bf16 and cast_act:
                # bf16 copy of q on ACT so the final mult runs 16-bit on DVE
                qb = bopool.tile([128, FD], BF16, tag="qb")
                nc.scalar.copy(qb[:, :], qt[:, :])
                qsrc = qb
            else:
                qsrc = qt
            qv = qsrc.rearrange("p (ts x) -> p ts x", ts=TS)
            ow = dst.rearrange("p (ts x) -> p ts x", ts=TS)
            tsplit = mult_split
            if probe == "halfmult":
                tsplit = 4
                nc.vector.tensor_tensor(
                    out=ow[:, 0:4, :],
                    in0=qv[:, 0:4, :],
                    in1=bass.AP(
                        tensor=sig.tensor, offset=sig.offset,
                        ap=[list(sig.ap[0]), [1, 4], [TS, J], [0, C]],
                    ),
                    op=Alu.mult,
                )
            elif tsplit > 0:
                nc.vector.tensor_tensor(
                    out=ow[:, 0:tsplit, :],
                    in0=qv[:, 0:tsplit, :],
                    in1=bass.AP(
                        tensor=sig.tensor, offset=sig.offset,
                        ap=[list(sig.ap[0]), [1, tsplit], [TS, J], [0, C]],
                    ),
                    op=Alu.mult,
                )
            if tsplit < TS:
                nc.gpsimd.tensor_tensor(
                    out=ow[:, tsplit:TS, :],
                    in0=qv[:, tsplit:TS, :],
                    in1=bass.AP(
                        tensor=sig.tensor, offset=sig.offset + tsplit,
                        ap=[list(sig.ap[0]), [1, TS - tsplit], [TS, J],
                            [0, C]],
                    ),
                    op=Alu.mult,
                )

            eng_out.dma_start(
                out=of[b, :].rearrange("(p x) -> p x", p=128), in_=dst[:, :]
            )

        if unroll:
            for _ in range(reps):
                emit_body()
        elif reps == 1:
            emit_body()
        else:
            assert reps % loop_unroll == 0
            with tc.For_i(0, reps // loop_unroll, 1,
                          staggered_reset=bool(staggered)):
                for _ in range(loop_unroll):
                    emit_body()
    return nc


def make_consts():
    smat = np.eye(128, k=1, dtype=np.float32)       # S[k, m] = 1 iff m == k+1
    pmat = np.triu(np.ones((128, 128), np.float32), k=1)  # strict prefix
    return smat, pmat


def make_in_maps(q, smat, pmat):
    return [
        {"q": q[c * BPC:(c + 1) * BPC], "smat": smat, "pmat": pmat}
        for c in range(NCORES)
    ]


def kernel(joint_rotations: np.ndarray) -> np.ndarray:
    q = np.ascontiguousarray(joint_rotations, dtype=np.float32)
    assert q.shape == (B, T, J, C)
    smat, pmat = make_consts()
    nc = build_nc()
    nc.finalize()   # run bacc passes (wait splitting, reg alloc) + freeze
    in_maps = make_in_maps(q, smat, pmat)
    res = run_bass_kernel_spmd(nc, in_maps, list(range(NCORES)))
    outs = [np.asarray(r["out"]).astype(np.float32) for r in res.results]
    return np.concatenate(outs, axis=0)

